# revision 12
# baseline (speedup 1.0000x reference)
"""Trainium2 Bass kernel for nn_CircuitBuilder (topk_masking).

Computes, for X [524288, 64] (f32), gate_weights [64, 130], output_weights
[64, 8], output_scale [8]:

    buf = [X | 0 | 1 | gate slots]
    top2[i] = top-2 of softmax(gate_weights[i, :66+i])   (data-independent
              of X; softmax is monotonic so = top-2 of masked logits)
    g_i = 1 - a*b  (continuous NAND chain, a/b gathered from buf)
    out = (gate_matrix @ output_weights) * output_scale

Strategy (pure data parallel over 8 NeuronCores, 65536 samples each):
  - The gate wiring is computed on host from gate_weights (tiny); the
    device kernel is built for that wiring.
  - Per-core layout: partition p owns 512 consecutive samples, processed
    as 2 supertiles of nsub=256 samples per partition. X tile is
    [128, nsub, 64] f32 (contiguous DMA); gate values live in a
    [128, 64, nsub] bf16 tile ("v" storage), where slot i holds
    v_i = alpha_i * (a_i*b_i) with a per-gate sign alpha chosen so each
    gate needs 1 fused DVE op (2 for gate×gate gates):
        m_i = a*b,  g_i = 1 - m_i,  v_i = alpha_i * m_i
    using scalar_tensor_tensor  out = (in0 op0 scalar) op1 in1.
  - Output: out = bias - sum_i W_i*m_i = bias + sum_i Wneg_i*v_i with
    Wneg_i = -alpha_i*W_i*scale. The v tile is rotated per 128-sample
    subtile with TensorE transposes into PSUM (packed bf16), drained to
    SBUF, then matmul'd against Wneg into [128, 8] psum chunks, bias
    added and DMA'd out.
"""

import hashlib
import sys
import types

import numpy as np
import ml_dtypes

N_SAMPLES = 524288
N_FEATURES = 64
N_GATES = 64
N_OUTPUTS = 8
BASE = N_FEATURES + 2            # 66
MAX_CONN = BASE + N_GATES        # 130
N_CORES = 8
N_LOC = N_SAMPLES // N_CORES     # 65536 samples per core
NSUB = 256                       # samples per partition per supertile
N_ST = N_LOC // (128 * NSUB)     # supertiles per core (2)


def _top2(gate_weights: np.ndarray) -> np.ndarray:
    """Top-2 connection indices per gate (matches jax.lax.top_k of the
    softmax: softmax is monotonic, top_k ties break to lower index,
    stable argsort of the negated row reproduces that)."""
    top2 = np.zeros((N_GATES, 2), dtype=np.int64)
    for i in range(N_GATES):
        row = np.asarray(gate_weights[i], dtype=np.float32).copy()
        row[BASE + i:] = -1e9
        top2[i] = np.argsort(-row, kind="stable")[:2]
    return top2


def _build_plan(gate_weights, output_weights, output_scale):
    """Host-side gate wiring -> per-gate op descriptors + output weights."""
    top2 = _top2(gate_weights)
    W = np.asarray(output_weights, dtype=np.float64)
    scale = np.asarray(output_scale, dtype=np.float64)

    ops = []          # list of dicts describing device ops per gate
    alpha = np.zeros(N_GATES, dtype=np.int64)
    for i in range(N_GATES):
        c0, c1 = int(top2[i][0]), int(top2[i][1])

        def kind(c):
            if c < N_FEATURES:
                return "x"
            if c == N_FEATURES:
                return "0"
            if c == N_FEATURES + 1:
                return "1"
            return "g"

        k0, k1 = kind(c0), kind(c1)
        # order canonically: g-operands first, then x, consts last
        pri = {"g": 0, "x": 1, "1": 2, "0": 3}
        if pri[k0] > pri[k1]:
            c0, c1, k0, k1 = c1, c0, k1, k0
        kk = k0 + k1
        if "0" in kk:
            ops.append({"op": "zero", "i": i})
            alpha[i] = 1
        elif kk == "x1":
            ops.append({"op": "copyx", "i": i, "c": c0})
            alpha[i] = 1
        elif kk == "g1":
            p = c0 - BASE
            ops.append({"op": "g1", "i": i, "p": p, "ap": int(alpha[p])})
            alpha[i] = 1
        elif kk == "xx":
            ops.append({"op": "xx", "i": i, "a": c0, "b": c1})
            alpha[i] = 1
        elif kk == "gx":
            p = c0 - BASE
            ap = int(alpha[p])
            ops.append({"op": "gx", "i": i, "p": p, "c": c1, "ap": ap})
            alpha[i] = -ap
        elif kk == "gg":
            p, q = c0 - BASE, c1 - BASE
            ap, aq = int(alpha[p]), int(alpha[q])
            if aq != -1 and ap == -1:
                p, q, ap, aq = q, p, aq, ap
            ops.append({"op": "gg", "i": i, "p": p, "q": q, "ap": ap, "aq": aq})
            alpha[i] = -ap if (aq == -1) else 1
        else:
            raise AssertionError(f"unexpected operand kinds {kk}")

    wneg = (-alpha[:, None] * W * scale[None, :]).astype(np.float64)
    bias = (W * scale[None, :]).sum(axis=0)
    return {
        "ops": ops,
        "alpha": alpha,
        "wneg_bf16": wneg.astype(ml_dtypes.bfloat16),
        "bias_f32": bias.astype(np.float32),
        "top2": top2,
    }


def _emulate_plan(plan, X):
    """Numpy emulation of the device program (bf16 v-storage) — used for
    host-side self-checks in development."""
    n = X.shape[0]
    bf = ml_dtypes.bfloat16
    V = np.zeros((N_GATES, n), dtype=bf)
    Xc = np.asarray(X, dtype=np.float32).T  # [64, n]
    for d in plan["ops"]:
        i = d["i"]
        if d["op"] == "zero":
            V[i] = 0
        elif d["op"] == "copyx":
            V[i] = Xc[d["c"]].astype(bf)
        elif d["op"] == "g1":
            V[i] = (V[d["p"]].astype(np.float32) * (-d["ap"]) + 1.0).astype(bf)
        elif d["op"] == "xx":
            V[i] = (Xc[d["a"]] * Xc[d["b"]]).astype(bf)
        elif d["op"] == "gx":
            cp = -d["ap"]
            V[i] = ((V[d["p"]].astype(np.float32) + cp) * Xc[d["c"]]).astype(bf)
        elif d["op"] == "gg":
            vp = V[d["p"]].astype(np.float32)
            vq = V[d["q"]].astype(np.float32)
            if d["aq"] == -1 or d["ap"] == -1:
                cp = -d["ap"]
                t = ((vp + cp) * vq).astype(bf).astype(np.float32)
                V[i] = ((vp + cp) + t).astype(bf)
            else:
                t = ((vp - 1.0) * vq).astype(bf).astype(np.float32)
                V[i] = ((t + 1.0) - vp).astype(bf)
    wneg = plan["wneg_bf16"].astype(np.float32)
    out = V.astype(np.float32).T @ wneg + plan["bias_f32"][None, :]
    return out


def _build_bass_kernel(plan, n_loc=N_LOC, nsub=NSUB):
    import concourse.bacc as bacc
    import concourse.tile as tile
    import concourse.mybir as mybir
    from concourse import masks

    f32 = mybir.dt.float32
    bf16 = mybir.dt.bfloat16
    mult = mybir.AluOpType.mult
    add = mybir.AluOpType.add
    subtract = mybir.AluOpType.subtract

    n_st = n_loc // (128 * nsub)
    assert n_st * 128 * nsub == n_loc

    nc = bacc.Bacc(None, target_bir_lowering=False)
    x_d = nc.dram_tensor("x", [n_loc, N_FEATURES], f32, kind="ExternalInput")
    wneg_d = nc.dram_tensor("wneg", [N_GATES, N_OUTPUTS], bf16,
                            kind="ExternalInput")
    brep_d = nc.dram_tensor("brep", [128, 512], f32, kind="ExternalInput")
    out_d = nc.dram_tensor("out", [n_loc, N_OUTPUTS], f32,
                           kind="ExternalOutput")

    # chunked views: 64-subtile chunks keep each DMA at 16KB/partition
    n_ch = nsub // 64
    xr = x_d.rearrange("(s p k j) c -> s k p j c",
                       s=n_st, p=128, k=n_ch, j=64)
    outr = out_d.rearrange("(s p k j) o -> s k p (j o)",
                           s=n_st, p=128, k=n_ch, j=64)

    with tile.TileContext(nc) as tc:
        with (
            tc.tile_pool(name="const", bufs=1) as cpool,
            tc.tile_pool(name="xp", bufs=2) as xpool,
            tc.tile_pool(name="vp", bufs=1) as vpool,
            tc.tile_pool(name="tp", bufs=4) as tpool,
            tc.tile_pool(name="vs", bufs=4) as vspool,
            tc.tile_pool(name="stg", bufs=2) as stgpool,
            tc.tile_pool(name="pt", bufs=2, space="PSUM") as ptpool,
            tc.tile_pool(name="po", bufs=2, space="PSUM") as popool,
        ):
            ident = cpool.tile([128, 128], bf16)
            masks.make_identity(nc, ident[:])
            wneg_sb = cpool.tile([N_GATES, N_OUTPUTS], bf16)
            nc.sync.dma_start(wneg_sb[:], wneg_d[:])
            brep_sb = cpool.tile([128, 512], f32)
            nc.sync.dma_start(brep_sb[:], brep_d[:])

            for st in range(n_st):
                xt = xpool.tile([128, nsub, N_FEATURES], f32)
                for k in range(n_ch):
                    nc.sync.dma_start(xt[:, k * 64:(k + 1) * 64, :], xr[st, k])
                vt = vpool.tile([128, N_GATES, nsub], bf16)

                def xcol(c):
                    return xt[:, :, c]

                def vslot(i):
                    return vt[:, i, :]

                for d in plan["ops"]:
                    i = d["i"]
                    o = d["op"]
                    if o == "zero":
                        nc.vector.memset(vslot(i), 0.0)
                    elif o == "copyx":
                        nc.vector.tensor_copy(vslot(i), xcol(d["c"]))
                    elif o == "g1":
                        nc.vector.tensor_scalar(
                            vslot(i), vslot(d["p"]),
                            float(-d["ap"]), 1.0, mult, add)
                    elif o == "xx":
                        nc.vector.tensor_mul(
                            vslot(i), xcol(d["a"]), xcol(d["b"]))
                    elif o == "gx":
                        nc.vector.scalar_tensor_tensor(
                            vslot(i), vslot(d["p"]), float(-d["ap"]),
                            xcol(d["c"]), add, mult)
                    elif o == "gg":
                        t = tpool.tile([128, nsub], bf16, tag="ggtmp")
                        if d["aq"] == -1 or d["ap"] == -1:
                            cp = float(-d["ap"])
                            nc.vector.scalar_tensor_tensor(
                                t[:], vslot(d["p"]), cp, vslot(d["q"]),
                                add, mult)
                            nc.vector.scalar_tensor_tensor(
                                vslot(i), vslot(d["p"]), cp, t[:],
                                add, add)
                        else:
                            nc.vector.scalar_tensor_tensor(
                                t[:], vslot(d["p"]), -1.0, vslot(d["q"]),
                                add, mult)
                            nc.vector.scalar_tensor_tensor(
                                vslot(i), t[:], 1.0, vslot(d["p"]),
                                add, subtract)

                # output: transpose v per 128-sample subtile, matmul with
                # wneg.  All PE operands stay at base partition 0 (matmuls
                # with base-64 operands flap the PE tile config and crash
                # at scale); 8 subtile-transposes pack one [64, 1024] psum
                # tile column-wise, drained alternately on ScalarE/DVE.
                stg = stgpool.tile([128, nsub * N_OUTPUTS], f32)
                for grp64 in range(nsub // 64):
                    po = popool.tile([128, 512], f32)
                    for g8 in range(8):
                        j0 = grp64 * 64 + g8 * 8
                        pt = ptpool.tile([64, 1024], bf16)
                        for jj in range(8):
                            nc.tensor.transpose(
                                pt[0:64, jj * 128:(jj + 1) * 128],
                                vt[:, :, j0 + jj], ident[:])
                        vs = vspool.tile([64, 1024], bf16)
                        if g8 % 2 == 0:
                            nc.scalar.copy(vs[:], pt[:])
                        else:
                            nc.vector.tensor_copy(vs[:], pt[:])
                        for jj in range(8):
                            jl = g8 * 8 + jj
                            nc.tensor.matmul(
                                po[:, jl * 8:jl * 8 + 8],
                                vs[0:64, jj * 128:(jj + 1) * 128],
                                wneg_sb[:], start=True, stop=True)
                    nc.vector.tensor_add(
                        stg[:, grp64 * 512:(grp64 + 1) * 512],
                        po[:], brep_sb[:])
                    nc.sync.dma_start(
                        outr[st, grp64],
                        stg[:, grp64 * 512:(grp64 + 1) * 512])

    nc.compile()
    return nc


_CACHE = {}


def _get_compiled(gate_weights, output_weights, output_scale):
    key = hashlib.sha256(
        np.asarray(gate_weights, np.float32).tobytes()
        + np.asarray(output_weights, np.float32).tobytes()
        + np.asarray(output_scale, np.float32).tobytes()
    ).hexdigest()
    if key not in _CACHE:
        plan = _build_plan(gate_weights, output_weights, output_scale)
        nc = _build_bass_kernel(plan)
        _CACHE[key] = (plan, nc)
    return _CACHE[key]


def kernel(X, gate_weights, output_weights, output_scale):
    X = np.asarray(X, dtype=np.float32)
    plan, nc = _get_compiled(gate_weights, output_weights, output_scale)

    brep = np.broadcast_to(
        np.tile(plan["bias_f32"], 64), (128, 512)).copy()
    in_maps = []
    for c in range(N_CORES):
        shard = X[c * N_LOC:(c + 1) * N_LOC]
        in_maps.append({
            "x": np.ascontiguousarray(shard),
            "wneg": plan["wneg_bf16"],
            "brep": brep,
        })

    from concourse.bass_utils import run_bass_kernel_spmd
    res = run_bass_kernel_spmd(nc, in_maps, list(range(N_CORES)))
    out = np.concatenate(
        [res.results[c]["out"] for c in range(N_CORES)], axis=0)
    return out.astype(np.float32)


# revision 27
# speedup vs baseline: 1.1890x; 1.1890x over previous
"""Trainium2 Bass kernel for nn_CircuitBuilder (topk_masking).

Computes, for X [524288, 64] (f32), gate_weights [64, 130], output_weights
[64, 8], output_scale [8]:

    buf = [X | 0 | 1 | gate slots]
    top2[i] = top-2 of softmax(gate_weights[i, :66+i])   (data-independent
              of X; softmax is monotonic so = top-2 of masked logits)
    g_i = 1 - a*b  (continuous NAND chain, a/b gathered from buf)
    out = (gate_matrix @ output_weights) * output_scale

Strategy (pure data parallel over 8 NeuronCores, 65536 samples each):
  - The gate wiring is computed on host from gate_weights (tiny); the
    device kernel is built for that wiring.
  - Per-core layout: partition p owns 512 consecutive samples, processed
    as 2 supertiles of nsub=256 samples per partition. X tile is
    [128, nsub, 64] f32 (contiguous DMA); gate values live in a
    [128, 64, nsub] bf16 tile ("v" storage), where slot i holds
    v_i = alpha_i * (a_i*b_i) with a per-gate sign alpha chosen so each
    gate needs 1 fused DVE op (2 for gate×gate gates):
        m_i = a*b,  g_i = 1 - m_i,  v_i = alpha_i * m_i
    using scalar_tensor_tensor  out = (in0 op0 scalar) op1 in1.
  - Output: out = bias - sum_i W_i*m_i = bias + sum_i Wneg_i*v_i with
    Wneg_i = -alpha_i*W_i*scale. The v tile is rotated per 128-sample
    subtile with TensorE transposes into PSUM (packed bf16), drained to
    SBUF, then matmul'd against Wneg into [128, 8] psum chunks, bias
    added and DMA'd out.
"""

import hashlib
import sys
import types

import numpy as np
import ml_dtypes

N_SAMPLES = 524288
N_FEATURES = 64
N_GATES = 64
N_OUTPUTS = 8
BASE = N_FEATURES + 2            # 66
MAX_CONN = BASE + N_GATES        # 130
N_CORES = 8
N_LOC = N_SAMPLES // N_CORES     # 65536 samples per core
NSUB = 256                       # samples per partition per supertile
N_ST = N_LOC // (128 * NSUB)     # supertiles per core (2)


def _top2(gate_weights: np.ndarray) -> np.ndarray:
    """Top-2 connection indices per gate (matches jax.lax.top_k of the
    softmax: softmax is monotonic, top_k ties break to lower index,
    stable argsort of the negated row reproduces that)."""
    top2 = np.zeros((N_GATES, 2), dtype=np.int64)
    for i in range(N_GATES):
        row = np.asarray(gate_weights[i], dtype=np.float32).copy()
        row[BASE + i:] = -1e9
        top2[i] = np.argsort(-row, kind="stable")[:2]
    return top2


def _build_plan(gate_weights, output_weights, output_scale):
    """Host-side gate wiring -> per-gate op descriptors + output weights."""
    top2 = _top2(gate_weights)
    W = np.asarray(output_weights, dtype=np.float64)
    scale = np.asarray(output_scale, dtype=np.float64)

    ops = []          # list of dicts describing device ops per gate
    alpha = np.zeros(N_GATES, dtype=np.int64)
    for i in range(N_GATES):
        c0, c1 = int(top2[i][0]), int(top2[i][1])

        def kind(c):
            if c < N_FEATURES:
                return "x"
            if c == N_FEATURES:
                return "0"
            if c == N_FEATURES + 1:
                return "1"
            return "g"

        k0, k1 = kind(c0), kind(c1)
        # order canonically: g-operands first, then x, consts last
        pri = {"g": 0, "x": 1, "1": 2, "0": 3}
        if pri[k0] > pri[k1]:
            c0, c1, k0, k1 = c1, c0, k1, k0
        kk = k0 + k1
        if "0" in kk:
            ops.append({"op": "zero", "i": i})
            alpha[i] = 1
        elif kk == "x1":
            ops.append({"op": "copyx", "i": i, "c": c0})
            alpha[i] = 1
        elif kk == "g1":
            p = c0 - BASE
            ops.append({"op": "g1", "i": i, "p": p, "ap": int(alpha[p])})
            alpha[i] = 1
        elif kk == "xx":
            ops.append({"op": "xx", "i": i, "a": c0, "b": c1})
            alpha[i] = 1
        elif kk == "gx":
            p = c0 - BASE
            ap = int(alpha[p])
            ops.append({"op": "gx", "i": i, "p": p, "c": c1, "ap": ap})
            alpha[i] = -ap
        elif kk == "gg":
            p, q = c0 - BASE, c1 - BASE
            ap, aq = int(alpha[p]), int(alpha[q])
            if aq != -1 and ap == -1:
                p, q, ap, aq = q, p, aq, ap
            ops.append({"op": "gg", "i": i, "p": p, "q": q, "ap": ap, "aq": aq})
            alpha[i] = -ap if (aq == -1) else 1
        else:
            raise AssertionError(f"unexpected operand kinds {kk}")

    wneg = (-alpha[:, None] * W * scale[None, :]).astype(np.float64)
    bias = (W * scale[None, :]).sum(axis=0)
    # block-diagonal-by-subtile-parity projection matrix for the
    # pair-transpose scheme: row (g*2 + jj), col (jj*8 + o) = wneg[g, o]
    wneg2 = np.zeros((2 * N_GATES, 2 * N_OUTPUTS), dtype=np.float64)
    for jj in range(2):
        wneg2[jj::2, jj * N_OUTPUTS:(jj + 1) * N_OUTPUTS] = wneg
    return {
        "ops": ops,
        "alpha": alpha,
        "wneg_bf16": wneg.astype(ml_dtypes.bfloat16),
        "wneg2_bf16": wneg2.astype(ml_dtypes.bfloat16),
        "bias_f32": bias.astype(np.float32),
        "top2": top2,
    }


def _emulate_plan(plan, X):
    """Numpy emulation of the device program (bf16 v-storage) — used for
    host-side self-checks in development."""
    n = X.shape[0]
    bf = ml_dtypes.bfloat16
    V = np.zeros((N_GATES, n), dtype=bf)
    Xc = np.asarray(X, dtype=np.float32).T  # [64, n]
    for d in plan["ops"]:
        i = d["i"]
        if d["op"] == "zero":
            V[i] = 0
        elif d["op"] == "copyx":
            V[i] = Xc[d["c"]].astype(bf)
        elif d["op"] == "g1":
            V[i] = (V[d["p"]].astype(np.float32) * (-d["ap"]) + 1.0).astype(bf)
        elif d["op"] == "xx":
            V[i] = (Xc[d["a"]] * Xc[d["b"]]).astype(bf)
        elif d["op"] == "gx":
            cp = -d["ap"]
            V[i] = ((V[d["p"]].astype(np.float32) + cp) * Xc[d["c"]]).astype(bf)
        elif d["op"] == "gg":
            vp = V[d["p"]].astype(np.float32)
            vq = V[d["q"]].astype(np.float32)
            if d["aq"] == -1 or d["ap"] == -1:
                cp = -d["ap"]
                t = ((vp + cp) * vq).astype(bf).astype(np.float32)
                V[i] = ((vp + cp) + t).astype(bf)
            else:
                t = ((vp - 1.0) * vq).astype(bf).astype(np.float32)
                V[i] = ((t + 1.0) - vp).astype(bf)
    wneg = plan["wneg_bf16"].astype(np.float32)
    out = V.astype(np.float32).T @ wneg + plan["bias_f32"][None, :]
    return out


def _build_bass_kernel(plan, n_loc=N_LOC, nsub=NSUB):
    import concourse.bacc as bacc
    import concourse.tile as tile
    import concourse.mybir as mybir
    from concourse import masks

    f32 = mybir.dt.float32
    bf16 = mybir.dt.bfloat16
    mult = mybir.AluOpType.mult
    add = mybir.AluOpType.add
    subtract = mybir.AluOpType.subtract

    n_st = n_loc // (128 * nsub)
    assert n_st * 128 * nsub == n_loc

    nc = bacc.Bacc(None, target_bir_lowering=False)
    x_d = nc.dram_tensor("x", [n_loc, N_FEATURES], f32, kind="ExternalInput")
    wneg2_d = nc.dram_tensor("wneg2", [2 * N_GATES, 2 * N_OUTPUTS], bf16,
                             kind="ExternalInput")
    bias_d = nc.dram_tensor("bias16", [2 * N_OUTPUTS, 1], f32,
                            kind="ExternalInput")
    # transposed output: row (jj*8 + o), col = (st, pair, p); host decodes
    out_d = nc.dram_tensor("out", [2 * N_OUTPUTS, n_loc // 2], f32,
                           kind="ExternalOutput")

    # chunked views: 64-subtile chunks keep each DMA at 16KB/partition
    n_ch = nsub // 64
    xr = x_d.rearrange("(s p k j) c -> s k p j c",
                       s=n_st, p=128, k=n_ch, j=64)
    outr = out_d.rearrange("r (s q pp) -> s r q pp",
                           s=n_st, q=nsub // 2, pp=128)

    with tile.TileContext(nc) as tc:
        with (
            tc.tile_pool(name="const", bufs=1) as cpool,
            tc.tile_pool(name="xp", bufs=2) as xpool,
            tc.tile_pool(name="vp", bufs=1) as vpool,
            tc.tile_pool(name="tp", bufs=4) as tpool,
            tc.tile_pool(name="vs", bufs=4) as vspool,
            tc.tile_pool(name="stg", bufs=2) as stgpool,
            tc.tile_pool(name="pt", bufs=2, space="PSUM") as ptpool,
            tc.tile_pool(name="po", bufs=2, space="PSUM") as popool,
        ):
            ident = cpool.tile([128, 128], bf16)
            masks.make_identity(nc, ident[:])
            wneg2_sb = cpool.tile([2 * N_GATES, 2 * N_OUTPUTS], bf16)
            nc.sync.dma_start(wneg2_sb[:], wneg2_d[:])
            bias_sb = cpool.tile([2 * N_OUTPUTS, 1], f32)
            nc.sync.dma_start(bias_sb[:], bias_d[:])

            for st in range(n_st):
                # x: [p, pair, jj, feature]; v: [p, pair, (gate*2 + jj)] so
                # each pair-block is contiguous (1-free-dim transpose input)
                xt = xpool.tile([128, nsub // 2, 2, N_FEATURES], f32)
                for k in range(n_ch):
                    nc.sync.dma_start(
                        xt[:, k * 32:(k + 1) * 32, :, :], xr[st, k])
                vt = vpool.tile([128, nsub // 2, 2 * N_GATES], bf16)

                def xcol(c):
                    return xt[:, :, :, c]

                def vslot(i):
                    return vt[:, :, 2 * i:2 * i + 2]

                for d in plan["ops"]:
                    i = d["i"]
                    o = d["op"]
                    if o == "zero":
                        nc.gpsimd.memset(vslot(i), 0.0)
                    elif o == "copyx":
                        nc.gpsimd.tensor_copy(vslot(i), xcol(d["c"]))
                    elif o == "g1":
                        nc.vector.tensor_scalar(
                            vslot(i), vslot(d["p"]),
                            float(-d["ap"]), 1.0, mult, add)
                    elif o == "xx":
                        # plain TT mult: GPSIMD runs these at ~DVE rate,
                        # freeing DVE for the stt ops it alone supports
                        nc.gpsimd.tensor_mul(
                            vslot(i), xcol(d["a"]), xcol(d["b"]))
                    elif o == "gx":
                        nc.vector.scalar_tensor_tensor(
                            vslot(i), vslot(d["p"]), float(-d["ap"]),
                            xcol(d["c"]), add, mult)
                    elif o == "gg":
                        t = tpool.tile([128, nsub // 2, 2], bf16, tag="ggtmp")
                        if d["aq"] == -1 or d["ap"] == -1:
                            cp = float(-d["ap"])
                            nc.vector.scalar_tensor_tensor(
                                t[:], vslot(d["p"]), cp, vslot(d["q"]),
                                add, mult)
                            nc.vector.scalar_tensor_tensor(
                                vslot(i), vslot(d["p"]), cp, t[:],
                                add, add)
                        else:
                            nc.vector.scalar_tensor_tensor(
                                t[:], vslot(d["p"]), -1.0, vslot(d["q"]),
                                add, mult)
                            nc.vector.scalar_tensor_tensor(
                                vslot(i), t[:], 1.0, vslot(d["p"]),
                                add, subtract)

                # output: transpose subtile PAIRS ([128, 64g x 2j] input ->
                # [128 rows=(g,jj), 128 samples]) and project with the
                # block-diagonal wneg2 (constant stationary).  All PE
                # operands stay at base partition 0 (matmuls with base-64
                # operands flap the PE tile config and crash at scale).
                # Host de-interleaves the [16, n] transposed output.
                for big in range(nsub // 32):     # 16 pairs per iteration
                    stg = stgpool.tile([2 * N_OUTPUTS, 16 * 128], f32)
                    for g8 in range(2):           # 8 pairs per pt bank
                        pt = ptpool.tile([128, 1024], bf16)
                        for c in range(8):
                            pr = big * 16 + g8 * 8 + c
                            nc.tensor.transpose(
                                pt[:, c * 128:(c + 1) * 128],
                                vt[:, pr, :], ident[:])
                        vs = vspool.tile([128, 1024], bf16)
                        if g8 == 0:
                            nc.scalar.copy(vs[:], pt[:])
                        else:
                            nc.vector.tensor_copy(vs[:], pt[:])
                        po = popool.tile([2 * N_OUTPUTS, 1024], f32)
                        for c in range(4):
                            nc.tensor.matmul(
                                po[:, c * 256:(c + 1) * 256],
                                wneg2_sb[:], vs[:, c * 256:(c + 1) * 256],
                                start=True, stop=True)
                        # drain + bias (per-partition scalar) fused
                        sslice = stg[:, g8 * 1024:(g8 + 1) * 1024]
                        if g8 == 0:
                            nc.vector.tensor_scalar(
                                sslice, po[:], bias_sb[:, 0:1], None, add)
                        else:
                            nc.scalar.activation(
                                sslice, po[:],
                                mybir.ActivationFunctionType.Identity,
                                bias=bias_sb[:, 0:1], scale=1.0)
                    nc.sync.dma_start(
                        outr[st, :, big * 16:(big + 1) * 16, :], stg[:])

    nc.compile()
    return nc


_CACHE = {}


def _get_compiled(gate_weights, output_weights, output_scale):
    key = hashlib.sha256(
        np.asarray(gate_weights, np.float32).tobytes()
        + np.asarray(output_weights, np.float32).tobytes()
        + np.asarray(output_scale, np.float32).tobytes()
    ).hexdigest()
    if key not in _CACHE:
        plan = _build_plan(gate_weights, output_weights, output_scale)
        nc = _build_bass_kernel(plan)
        _CACHE[key] = (plan, nc)
    return _CACHE[key]


def _decode_out(dev_out, plan, n_loc=N_LOC, nsub=NSUB):
    """[16, n_loc//2] transposed device output (bias included) ->
    [n_loc, 8]."""
    n_st = n_loc // (128 * nsub)
    o5 = np.asarray(dev_out).reshape(2, N_OUTPUTS, n_st, nsub // 2, 128)
    # [jj, o, st, pr, p] -> [st, p, pr, jj, o]
    return np.transpose(o5, (2, 4, 3, 0, 1)).reshape(n_loc, N_OUTPUTS)


def make_in_maps(X, plan, n_loc=N_LOC, n_cores=N_CORES):
    bias16 = np.concatenate([plan["bias_f32"], plan["bias_f32"]])
    bias16 = bias16.reshape(2 * N_OUTPUTS, 1).astype(np.float32)
    in_maps = []
    for c in range(n_cores):
        shard = X[c * n_loc:(c + 1) * n_loc]
        in_maps.append({
            "x": np.ascontiguousarray(shard),
            "wneg2": plan["wneg2_bf16"],
            "bias16": bias16,
        })
    return in_maps


def kernel(X, gate_weights, output_weights, output_scale):
    X = np.asarray(X, dtype=np.float32)
    plan, nc = _get_compiled(gate_weights, output_weights, output_scale)
    in_maps = make_in_maps(X, plan)

    from concourse.bass_utils import run_bass_kernel_spmd
    res = run_bass_kernel_spmd(nc, in_maps, list(range(N_CORES)))
    out = np.concatenate(
        [_decode_out(res.results[c]["out"], plan) for c in range(N_CORES)],
        axis=0)
    return out.astype(np.float32)


# revision 28
# speedup vs baseline: 1.3624x; 1.1458x over previous
"""Trainium2 Bass kernel for nn_CircuitBuilder (topk_masking).

Computes, for X [524288, 64] (f32), gate_weights [64, 130], output_weights
[64, 8], output_scale [8]:

    buf = [X | 0 | 1 | gate slots]
    top2[i] = top-2 of softmax(gate_weights[i, :66+i])   (data-independent
              of X; softmax is monotonic so = top-2 of masked logits)
    g_i = 1 - a*b  (continuous NAND chain, a/b gathered from buf)
    out = (gate_matrix @ output_weights) * output_scale

Strategy (pure data parallel over 8 NeuronCores, 65536 samples each):
  - The gate wiring is computed on host from gate_weights (tiny); the
    device kernel is built for that wiring.
  - Per-core layout: partition p owns 512 consecutive samples, processed
    as 2 supertiles of nsub=256 samples per partition. X tile is
    [128, nsub, 64] f32 (contiguous DMA); gate values live in a
    [128, 64, nsub] bf16 tile ("v" storage), where slot i holds
    v_i = alpha_i * (a_i*b_i) with a per-gate sign alpha chosen so each
    gate needs 1 fused DVE op (2 for gate×gate gates):
        m_i = a*b,  g_i = 1 - m_i,  v_i = alpha_i * m_i
    using scalar_tensor_tensor  out = (in0 op0 scalar) op1 in1.
  - Output: out = bias - sum_i W_i*m_i = bias + sum_i Wneg_i*v_i with
    Wneg_i = -alpha_i*W_i*scale. The v tile is rotated per 128-sample
    subtile with TensorE transposes into PSUM (packed bf16), drained to
    SBUF, then matmul'd against Wneg into [128, 8] psum chunks, bias
    added and DMA'd out.
"""

import hashlib
import sys
import types

import numpy as np
import ml_dtypes

N_SAMPLES = 524288
N_FEATURES = 64
N_GATES = 64
N_OUTPUTS = 8
BASE = N_FEATURES + 2            # 66
MAX_CONN = BASE + N_GATES        # 130
N_CORES = 8
N_LOC = N_SAMPLES // N_CORES     # 65536 samples per core
NSUB = 256                       # samples per partition per supertile
N_ST = N_LOC // (128 * NSUB)     # supertiles per core (2)


def _top2(gate_weights: np.ndarray) -> np.ndarray:
    """Top-2 connection indices per gate (matches jax.lax.top_k of the
    softmax: softmax is monotonic, top_k ties break to lower index,
    stable argsort of the negated row reproduces that)."""
    top2 = np.zeros((N_GATES, 2), dtype=np.int64)
    for i in range(N_GATES):
        row = np.asarray(gate_weights[i], dtype=np.float32).copy()
        row[BASE + i:] = -1e9
        top2[i] = np.argsort(-row, kind="stable")[:2]
    return top2


def _build_plan(gate_weights, output_weights, output_scale):
    """Host-side gate wiring -> per-gate op descriptors + output weights."""
    top2 = _top2(gate_weights)
    W = np.asarray(output_weights, dtype=np.float64)
    scale = np.asarray(output_scale, dtype=np.float64)

    ops = []          # list of dicts describing device ops per gate
    alpha = np.zeros(N_GATES, dtype=np.int64)
    for i in range(N_GATES):
        c0, c1 = int(top2[i][0]), int(top2[i][1])

        def kind(c):
            if c < N_FEATURES:
                return "x"
            if c == N_FEATURES:
                return "0"
            if c == N_FEATURES + 1:
                return "1"
            return "g"

        k0, k1 = kind(c0), kind(c1)
        # order canonically: g-operands first, then x, consts last
        pri = {"g": 0, "x": 1, "1": 2, "0": 3}
        if pri[k0] > pri[k1]:
            c0, c1, k0, k1 = c1, c0, k1, k0
        kk = k0 + k1
        if "0" in kk:
            ops.append({"op": "zero", "i": i})
            alpha[i] = 1
        elif kk == "x1":
            ops.append({"op": "copyx", "i": i, "c": c0})
            alpha[i] = 1
        elif kk == "g1":
            p = c0 - BASE
            ops.append({"op": "g1", "i": i, "p": p, "ap": int(alpha[p])})
            alpha[i] = 1
        elif kk == "xx":
            ops.append({"op": "xx", "i": i, "a": c0, "b": c1})
            alpha[i] = 1
        elif kk == "gx":
            p = c0 - BASE
            ap = int(alpha[p])
            ops.append({"op": "gx", "i": i, "p": p, "c": c1, "ap": ap})
            alpha[i] = -ap
        elif kk == "gg":
            p, q = c0 - BASE, c1 - BASE
            ap, aq = int(alpha[p]), int(alpha[q])
            if aq != -1 and ap == -1:
                p, q, ap, aq = q, p, aq, ap
            ops.append({"op": "gg", "i": i, "p": p, "q": q, "ap": ap, "aq": aq})
            alpha[i] = -ap if (aq == -1) else 1
        else:
            raise AssertionError(f"unexpected operand kinds {kk}")

    wneg = (-alpha[:, None] * W * scale[None, :]).astype(np.float64)
    bias = (W * scale[None, :]).sum(axis=0)
    # block-diagonal-by-subtile-parity projection matrix for the
    # pair-transpose scheme: row (g*2 + jj), col (jj*8 + o) = wneg[g, o]
    wneg2 = np.zeros((2 * N_GATES, 2 * N_OUTPUTS), dtype=np.float64)
    for jj in range(2):
        wneg2[jj::2, jj * N_OUTPUTS:(jj + 1) * N_OUTPUTS] = wneg
    return {
        "ops": ops,
        "alpha": alpha,
        "wneg_bf16": wneg.astype(ml_dtypes.bfloat16),
        "wneg2_bf16": wneg2.astype(ml_dtypes.bfloat16),
        "bias_f32": bias.astype(np.float32),
        "top2": top2,
    }


def _emulate_plan(plan, X):
    """Numpy emulation of the device program (bf16 v-storage) — used for
    host-side self-checks in development."""
    n = X.shape[0]
    bf = ml_dtypes.bfloat16
    V = np.zeros((N_GATES, n), dtype=bf)
    Xc = np.asarray(X, dtype=np.float32).T  # [64, n]
    for d in plan["ops"]:
        i = d["i"]
        if d["op"] == "zero":
            V[i] = 0
        elif d["op"] == "copyx":
            V[i] = Xc[d["c"]].astype(bf)
        elif d["op"] == "g1":
            V[i] = (V[d["p"]].astype(np.float32) * (-d["ap"]) + 1.0).astype(bf)
        elif d["op"] == "xx":
            V[i] = (Xc[d["a"]] * Xc[d["b"]]).astype(bf)
        elif d["op"] == "gx":
            cp = -d["ap"]
            V[i] = ((V[d["p"]].astype(np.float32) + cp) * Xc[d["c"]]).astype(bf)
        elif d["op"] == "gg":
            vp = V[d["p"]].astype(np.float32)
            vq = V[d["q"]].astype(np.float32)
            if d["aq"] == -1 or d["ap"] == -1:
                cp = -d["ap"]
                t = ((vp + cp) * vq).astype(bf).astype(np.float32)
                V[i] = ((vp + cp) + t).astype(bf)
            else:
                t = ((vp - 1.0) * vq).astype(bf).astype(np.float32)
                V[i] = ((t + 1.0) - vp).astype(bf)
    wneg = plan["wneg_bf16"].astype(np.float32)
    out = V.astype(np.float32).T @ wneg + plan["bias_f32"][None, :]
    return out


def _build_bass_kernel(plan, n_loc=N_LOC, nsub=NSUB):
    import concourse.bacc as bacc
    import concourse.tile as tile
    import concourse.mybir as mybir
    from concourse import masks

    f32 = mybir.dt.float32
    bf16 = mybir.dt.bfloat16
    mult = mybir.AluOpType.mult
    add = mybir.AluOpType.add
    subtract = mybir.AluOpType.subtract

    n_st = n_loc // (128 * nsub)
    assert n_st * 128 * nsub == n_loc

    nc = bacc.Bacc(None, target_bir_lowering=False)
    x_d = nc.dram_tensor("x", [n_loc, N_FEATURES], f32, kind="ExternalInput")
    wneg2_d = nc.dram_tensor("wneg2", [2 * N_GATES, 2 * N_OUTPUTS], bf16,
                             kind="ExternalInput")
    bias_d = nc.dram_tensor("bias16", [2 * N_OUTPUTS, 1], f32,
                            kind="ExternalInput")
    # transposed output: row (jj*8 + o), col = (st, pair, p); host decodes
    out_d = nc.dram_tensor("out", [2 * N_OUTPUTS, n_loc // 2], f32,
                           kind="ExternalOutput")

    # chunked views: 64-subtile chunks keep each DMA at 16KB/partition
    n_ch = nsub // 64
    xr = x_d.rearrange("(s p k j) c -> s k p j c",
                       s=n_st, p=128, k=n_ch, j=64)
    outr = out_d.rearrange("r (s q pp) -> s r q pp",
                           s=n_st, q=nsub // 2, pp=128)

    with tile.TileContext(nc) as tc:
        with (
            tc.tile_pool(name="const", bufs=1) as cpool,
            tc.tile_pool(name="xp", bufs=2) as xpool,
            tc.tile_pool(name="vp", bufs=1) as vpool,
            tc.tile_pool(name="tp", bufs=4) as tpool,
            tc.tile_pool(name="vs", bufs=4) as vspool,
            tc.tile_pool(name="stg", bufs=2) as stgpool,
            tc.tile_pool(name="pt", bufs=2, space="PSUM") as ptpool,
            tc.tile_pool(name="po", bufs=2, space="PSUM") as popool,
        ):
            ident = cpool.tile([128, 128], bf16)
            masks.make_identity(nc, ident[:])
            wneg2_sb = cpool.tile([2 * N_GATES, 2 * N_OUTPUTS], bf16)
            nc.sync.dma_start(wneg2_sb[:], wneg2_d[:])
            bias_sb = cpool.tile([2 * N_OUTPUTS, 1], f32)
            nc.sync.dma_start(bias_sb[:], bias_d[:])

            for st in range(n_st):
                # x: [p, pair, jj, feature]; v: [p, pair, (gate*2 + jj)] so
                # each pair-block is contiguous (1-free-dim transpose input)
                xt = xpool.tile([128, nsub // 2, 2, N_FEATURES], f32)
                for k in range(n_ch):
                    nc.sync.dma_start(
                        xt[:, k * 32:(k + 1) * 32, :, :], xr[st, k])
                vt = vpool.tile([128, nsub // 2, 2 * N_GATES], bf16)

                def xcol(c):
                    return xt[:, :, :, c]

                def vslot(i):
                    return vt[:, :, 2 * i:2 * i + 2]

                for d in plan["ops"]:
                    i = d["i"]
                    o = d["op"]
                    if o == "zero":
                        nc.vector.memset(vslot(i), 0.0)
                    elif o == "copyx":
                        nc.vector.tensor_copy(vslot(i), xcol(d["c"]))
                    elif o == "g1":
                        nc.vector.tensor_scalar(
                            vslot(i), vslot(d["p"]),
                            float(-d["ap"]), 1.0, mult, add)
                    elif o == "xx":
                        # stt form measures faster than plain TENSOR_TENSOR
                        nc.vector.scalar_tensor_tensor(
                            vslot(i), xcol(d["a"]), 1.0,
                            xcol(d["b"]), mult, mult)
                    elif o == "gx":
                        nc.vector.scalar_tensor_tensor(
                            vslot(i), vslot(d["p"]), float(-d["ap"]),
                            xcol(d["c"]), add, mult)
                    elif o == "gg":
                        t = tpool.tile([128, nsub // 2, 2], bf16, tag="ggtmp")
                        if d["aq"] == -1 or d["ap"] == -1:
                            cp = float(-d["ap"])
                            nc.vector.scalar_tensor_tensor(
                                t[:], vslot(d["p"]), cp, vslot(d["q"]),
                                add, mult)
                            nc.vector.scalar_tensor_tensor(
                                vslot(i), vslot(d["p"]), cp, t[:],
                                add, add)
                        else:
                            nc.vector.scalar_tensor_tensor(
                                t[:], vslot(d["p"]), -1.0, vslot(d["q"]),
                                add, mult)
                            nc.vector.scalar_tensor_tensor(
                                vslot(i), t[:], 1.0, vslot(d["p"]),
                                add, subtract)

                # output: transpose subtile PAIRS ([128, 64g x 2j] input ->
                # [128 rows=(g,jj), 128 samples]) and project with the
                # block-diagonal wneg2 (constant stationary).  All PE
                # operands stay at base partition 0 (matmuls with base-64
                # operands flap the PE tile config and crash at scale).
                # Host de-interleaves the [16, n] transposed output.
                for big in range(nsub // 32):     # 16 pairs per iteration
                    stg = stgpool.tile([2 * N_OUTPUTS, 16 * 128], f32)
                    for g8 in range(2):           # 8 pairs per pt bank
                        pt = ptpool.tile([128, 1024], bf16)
                        for c in range(8):
                            pr = big * 16 + g8 * 8 + c
                            nc.tensor.transpose(
                                pt[:, c * 128:(c + 1) * 128],
                                vt[:, pr, :], ident[:])
                        vs = vspool.tile([128, 1024], bf16)
                        if g8 == 0:
                            nc.scalar.copy(vs[:], pt[:])
                        else:
                            nc.vector.tensor_copy(vs[:], pt[:])
                        po = popool.tile([2 * N_OUTPUTS, 1024], f32)
                        for c in range(4):
                            nc.tensor.matmul(
                                po[:, c * 256:(c + 1) * 256],
                                wneg2_sb[:], vs[:, c * 256:(c + 1) * 256],
                                start=True, stop=True)
                        # drain + bias (per-partition scalar) fused
                        sslice = stg[:, g8 * 1024:(g8 + 1) * 1024]
                        if g8 == 0:
                            nc.vector.tensor_scalar(
                                sslice, po[:], bias_sb[:, 0:1], None, add)
                        else:
                            nc.scalar.activation(
                                sslice, po[:],
                                mybir.ActivationFunctionType.Identity,
                                bias=bias_sb[:, 0:1], scale=1.0)
                    nc.sync.dma_start(
                        outr[st, :, big * 16:(big + 1) * 16, :], stg[:])

    nc.compile()
    return nc


_CACHE = {}


def _get_compiled(gate_weights, output_weights, output_scale):
    key = hashlib.sha256(
        np.asarray(gate_weights, np.float32).tobytes()
        + np.asarray(output_weights, np.float32).tobytes()
        + np.asarray(output_scale, np.float32).tobytes()
    ).hexdigest()
    if key not in _CACHE:
        plan = _build_plan(gate_weights, output_weights, output_scale)
        nc = _build_bass_kernel(plan)
        _CACHE[key] = (plan, nc)
    return _CACHE[key]


def _decode_out(dev_out, plan, n_loc=N_LOC, nsub=NSUB):
    """[16, n_loc//2] transposed device output (bias included) ->
    [n_loc, 8]."""
    n_st = n_loc // (128 * nsub)
    o5 = np.asarray(dev_out).reshape(2, N_OUTPUTS, n_st, nsub // 2, 128)
    # [jj, o, st, pr, p] -> [st, p, pr, jj, o]
    return np.transpose(o5, (2, 4, 3, 0, 1)).reshape(n_loc, N_OUTPUTS)


def make_in_maps(X, plan, n_loc=N_LOC, n_cores=N_CORES):
    bias16 = np.concatenate([plan["bias_f32"], plan["bias_f32"]])
    bias16 = bias16.reshape(2 * N_OUTPUTS, 1).astype(np.float32)
    in_maps = []
    for c in range(n_cores):
        shard = X[c * n_loc:(c + 1) * n_loc]
        in_maps.append({
            "x": np.ascontiguousarray(shard),
            "wneg2": plan["wneg2_bf16"],
            "bias16": bias16,
        })
    return in_maps


def kernel(X, gate_weights, output_weights, output_scale):
    X = np.asarray(X, dtype=np.float32)
    plan, nc = _get_compiled(gate_weights, output_weights, output_scale)
    in_maps = make_in_maps(X, plan)

    from concourse.bass_utils import run_bass_kernel_spmd
    res = run_bass_kernel_spmd(nc, in_maps, list(range(N_CORES)))
    out = np.concatenate(
        [_decode_out(res.results[c]["out"], plan) for c in range(N_CORES)],
        axis=0)
    return out.astype(np.float32)


# revision 33
# speedup vs baseline: 1.8037x; 1.3239x over previous
"""Trainium2 Bass kernel for nn_CircuitBuilder (topk_masking).

Computes, for X [524288, 64] (f32), gate_weights [64, 130], output_weights
[64, 8], output_scale [8]:

    buf = [X | 0 | 1 | gate slots]
    top2[i] = top-2 of softmax(gate_weights[i, :66+i])   (data-independent
              of X; softmax is monotonic so = top-2 of masked logits)
    g_i = 1 - a*b  (continuous NAND chain, a/b gathered from buf)
    out = (gate_matrix @ output_weights) * output_scale

Strategy (pure data parallel over 8 NeuronCores, 65536 samples each):
  - The gate wiring is computed on host from gate_weights (tiny); the
    device kernel is built for that wiring.
  - Per-core layout: partition p owns 512 consecutive samples, processed
    as 2 supertiles of nsub=256 samples per partition. X tile is
    [128, nsub, 64] f32 (contiguous DMA); gate values live in a
    [128, 64, nsub] bf16 tile ("v" storage), where slot i holds
    v_i = alpha_i * (a_i*b_i) with a per-gate sign alpha chosen so each
    gate needs 1 fused DVE op (2 for gate×gate gates):
        m_i = a*b,  g_i = 1 - m_i,  v_i = alpha_i * m_i
    using scalar_tensor_tensor  out = (in0 op0 scalar) op1 in1.
  - Output: out = bias - sum_i W_i*m_i = bias + sum_i Wneg_i*v_i with
    Wneg_i = -alpha_i*W_i*scale. The v tile is rotated per 128-sample
    subtile with TensorE transposes into PSUM (packed bf16), drained to
    SBUF, then matmul'd against Wneg into [128, 8] psum chunks, bias
    added and DMA'd out.
"""

import hashlib
import sys
import types

import numpy as np
import ml_dtypes

N_SAMPLES = 524288
N_FEATURES = 64
N_GATES = 64
N_OUTPUTS = 8
BASE = N_FEATURES + 2            # 66
MAX_CONN = BASE + N_GATES        # 130
N_CORES = 8
N_LOC = N_SAMPLES // N_CORES     # 65536 samples per core
NSUB = 256                       # samples per partition per supertile
N_ST = N_LOC // (128 * NSUB)     # supertiles per core (2)


def _top2(gate_weights: np.ndarray) -> np.ndarray:
    """Top-2 connection indices per gate (matches jax.lax.top_k of the
    softmax: softmax is monotonic, top_k ties break to lower index,
    stable argsort of the negated row reproduces that)."""
    top2 = np.zeros((N_GATES, 2), dtype=np.int64)
    for i in range(N_GATES):
        row = np.asarray(gate_weights[i], dtype=np.float32).copy()
        row[BASE + i:] = -1e9
        top2[i] = np.argsort(-row, kind="stable")[:2]
    return top2


def _build_plan(gate_weights, output_weights, output_scale):
    """Host-side gate wiring -> per-gate op descriptors + output weights."""
    top2 = _top2(gate_weights)
    W = np.asarray(output_weights, dtype=np.float64)
    scale = np.asarray(output_scale, dtype=np.float64)

    ops = []          # list of dicts describing device ops per gate
    alpha = np.zeros(N_GATES, dtype=np.int64)
    for i in range(N_GATES):
        c0, c1 = int(top2[i][0]), int(top2[i][1])

        def kind(c):
            if c < N_FEATURES:
                return "x"
            if c == N_FEATURES:
                return "0"
            if c == N_FEATURES + 1:
                return "1"
            return "g"

        k0, k1 = kind(c0), kind(c1)
        # order canonically: g-operands first, then x, consts last
        pri = {"g": 0, "x": 1, "1": 2, "0": 3}
        if pri[k0] > pri[k1]:
            c0, c1, k0, k1 = c1, c0, k1, k0
        kk = k0 + k1
        if "0" in kk:
            ops.append({"op": "zero", "i": i})
            alpha[i] = 1
        elif kk == "x1":
            ops.append({"op": "copyx", "i": i, "c": c0})
            alpha[i] = 1
        elif kk == "g1":
            p = c0 - BASE
            ops.append({"op": "g1", "i": i, "p": p, "ap": int(alpha[p])})
            alpha[i] = 1
        elif kk == "xx":
            ops.append({"op": "xx", "i": i, "a": c0, "b": c1})
            alpha[i] = 1
        elif kk == "gx":
            p = c0 - BASE
            ap = int(alpha[p])
            ops.append({"op": "gx", "i": i, "p": p, "c": c1, "ap": ap})
            alpha[i] = -ap
        elif kk == "gg":
            p, q = c0 - BASE, c1 - BASE
            ap, aq = int(alpha[p]), int(alpha[q])
            if aq != -1 and ap == -1:
                p, q, ap, aq = q, p, aq, ap
            ops.append({"op": "gg", "i": i, "p": p, "q": q, "ap": ap, "aq": aq})
            alpha[i] = -ap if (aq == -1) else 1
        else:
            raise AssertionError(f"unexpected operand kinds {kk}")

    wneg = (-alpha[:, None] * W * scale[None, :]).astype(np.float64)
    bias = (W * scale[None, :]).sum(axis=0)
    # block-diagonal-by-subtile-parity projection matrix for the
    # pair-transpose scheme: row (g*2 + jj), col (jj*8 + o) = wneg[g, o]
    wneg2 = np.zeros((2 * N_GATES, 2 * N_OUTPUTS), dtype=np.float64)
    for jj in range(2):
        wneg2[jj::2, jj * N_OUTPUTS:(jj + 1) * N_OUTPUTS] = wneg
    return {
        "ops": ops,
        "alpha": alpha,
        "wneg_bf16": wneg.astype(ml_dtypes.bfloat16),
        "wneg2_bf16": wneg2.astype(ml_dtypes.bfloat16),
        "bias_f32": bias.astype(np.float32),
        "top2": top2,
    }


def _emulate_plan(plan, X):
    """Numpy emulation of the device program (bf16 v-storage) — used for
    host-side self-checks in development."""
    n = X.shape[0]
    bf = ml_dtypes.bfloat16
    V = np.zeros((N_GATES, n), dtype=bf)
    # device receives X pre-quantized to bf16
    Xc = np.asarray(X, dtype=np.float32).T.astype(bf).astype(np.float32)
    for d in plan["ops"]:
        i = d["i"]
        if d["op"] == "zero":
            V[i] = 0
        elif d["op"] == "copyx":
            V[i] = Xc[d["c"]].astype(bf)
        elif d["op"] == "g1":
            V[i] = (V[d["p"]].astype(np.float32) * (-d["ap"]) + 1.0).astype(bf)
        elif d["op"] == "xx":
            V[i] = (Xc[d["a"]] * Xc[d["b"]]).astype(bf)
        elif d["op"] == "gx":
            cp = -d["ap"]
            V[i] = ((V[d["p"]].astype(np.float32) + cp) * Xc[d["c"]]).astype(bf)
        elif d["op"] == "gg":
            vp = V[d["p"]].astype(np.float32)
            vq = V[d["q"]].astype(np.float32)
            if d["aq"] == -1 or d["ap"] == -1:
                cp = -d["ap"]
                t = ((vp + cp) * vq).astype(bf).astype(np.float32)
                V[i] = ((vp + cp) + t).astype(bf)
            else:
                t = ((vp - 1.0) * vq).astype(bf).astype(np.float32)
                V[i] = ((t + 1.0) - vp).astype(bf)
    wneg = plan["wneg_bf16"].astype(np.float32)
    out = V.astype(np.float32).T @ wneg + plan["bias_f32"][None, :]
    return out


def _build_bass_kernel(plan, n_loc=N_LOC, nsub=NSUB):
    import concourse.bacc as bacc
    import concourse.tile as tile
    import concourse.mybir as mybir
    from concourse import masks

    f32 = mybir.dt.float32
    bf16 = mybir.dt.bfloat16
    mult = mybir.AluOpType.mult
    add = mybir.AluOpType.add
    subtract = mybir.AluOpType.subtract

    n_st = n_loc // (128 * nsub)
    assert n_st * 128 * nsub == n_loc

    nc = bacc.Bacc(None, target_bir_lowering=False)
    # x pre-transposed on host to slot-major bf16:
    # xg[st, p, c, j] = X[st*128*nsub + p*nsub + j, c]
    x_d = nc.dram_tensor("xg", [n_st, 128, N_FEATURES, nsub], bf16,
                         kind="ExternalInput")
    wneg2_d = nc.dram_tensor("wneg2", [2 * N_GATES, 2 * N_OUTPUTS], bf16,
                             kind="ExternalInput")
    bias_d = nc.dram_tensor("bias16", [2 * N_OUTPUTS, 1], f32,
                            kind="ExternalInput")
    # transposed output: row (jj*8 + o), col = (st, pair, p); host decodes
    out_d = nc.dram_tensor("out", [2 * N_OUTPUTS, n_loc // 2], f32,
                           kind="ExternalOutput")

    outr = out_d.rearrange("r (s q pp) -> s r q pp",
                           s=n_st, q=nsub // 2, pp=128)

    with tile.TileContext(nc) as tc:
        with (
            tc.tile_pool(name="const", bufs=1) as cpool,
            tc.tile_pool(name="xp", bufs=2) as xpool,
            tc.tile_pool(name="vp", bufs=2) as vpool,
            tc.tile_pool(name="tp", bufs=4) as tpool,
            tc.tile_pool(name="vs", bufs=4) as vspool,
            tc.tile_pool(name="stg", bufs=2) as stgpool,
            tc.tile_pool(name="pt", bufs=3, space="PSUM") as ptpool,
            tc.tile_pool(name="po", bufs=2, space="PSUM") as popool,
        ):
            ident = cpool.tile([128, 128], bf16)
            masks.make_identity(nc, ident[:])
            wneg2_sb = cpool.tile([2 * N_GATES, 2 * N_OUTPUTS], bf16)
            nc.sync.dma_start(wneg2_sb[:], wneg2_d[:])
            bias_sb = cpool.tile([2 * N_OUTPUTS, 1], f32)
            nc.sync.dma_start(bias_sb[:], bias_d[:])

            for st in range(n_st):
                # x: [p, col, pair, jj] slot-major bf16 (host-transposed);
                # v: [p, pair, (gate*2 + jj)] so each pair-block is
                # contiguous (1-free-dim transpose input)
                xt = xpool.tile([128, N_FEATURES, nsub // 2, 2], bf16)
                for k in range(2):
                    nc.sync.dma_start(
                        xt[:, k * 32:(k + 1) * 32, :, :],
                        x_d[st, :, k * 32:(k + 1) * 32, :].rearrange(
                            "p c (q t) -> p c q t", t=2))
                vt = vpool.tile([128, nsub // 2, 2 * N_GATES], bf16)

                def xcol(c):
                    return xt[:, c, :, :]

                def vslot(i):
                    return vt[:, :, 2 * i:2 * i + 2]

                for d in plan["ops"]:
                    i = d["i"]
                    o = d["op"]
                    if o == "zero":
                        nc.vector.memset(vslot(i), 0.0)
                    elif o == "copyx":
                        nc.vector.tensor_copy(vslot(i), xcol(d["c"]))
                    elif o == "g1":
                        nc.vector.tensor_scalar(
                            vslot(i), vslot(d["p"]),
                            float(-d["ap"]), 1.0, mult, add)
                    elif o == "xx":
                        # stt form measures faster than plain TENSOR_TENSOR
                        nc.vector.scalar_tensor_tensor(
                            vslot(i), xcol(d["a"]), 1.0,
                            xcol(d["b"]), mult, mult)
                    elif o == "gx":
                        nc.vector.scalar_tensor_tensor(
                            vslot(i), vslot(d["p"]), float(-d["ap"]),
                            xcol(d["c"]), add, mult)
                    elif o == "gg":
                        t = tpool.tile([128, nsub // 2, 2], bf16, tag="ggtmp")
                        if d["aq"] == -1 or d["ap"] == -1:
                            cp = float(-d["ap"])
                            nc.vector.scalar_tensor_tensor(
                                t[:], vslot(d["p"]), cp, vslot(d["q"]),
                                add, mult)
                            nc.vector.scalar_tensor_tensor(
                                vslot(i), vslot(d["p"]), cp, t[:],
                                add, add)
                        else:
                            nc.vector.scalar_tensor_tensor(
                                t[:], vslot(d["p"]), -1.0, vslot(d["q"]),
                                add, mult)
                            nc.vector.scalar_tensor_tensor(
                                vslot(i), t[:], 1.0, vslot(d["p"]),
                                add, subtract)

                # output: transpose subtile PAIRS ([128, 64g x 2j] input ->
                # [128 rows=(g,jj), 128 samples]) and project with the
                # block-diagonal wneg2 (constant stationary).  All PE
                # operands stay at base partition 0 (matmuls with base-64
                # operands flap the PE tile config and crash at scale).
                # Host de-interleaves the [16, n] transposed output.
                for big in range(nsub // 32):     # 16 pairs per iteration
                    stg = stgpool.tile([2 * N_OUTPUTS, 16 * 128], f32)
                    for g8 in range(2):           # 8 pairs per pt bank
                        pt = ptpool.tile([128, 1024], bf16)
                        for c in range(8):
                            pr = big * 16 + g8 * 8 + c
                            nc.tensor.transpose(
                                pt[:, c * 128:(c + 1) * 128],
                                vt[:, pr, :], ident[:])
                        vs = vspool.tile([128, 1024], bf16)
                        if g8 == 0:
                            nc.scalar.copy(vs[:], pt[:])
                        else:
                            nc.vector.tensor_copy(vs[:], pt[:])
                        po = popool.tile([2 * N_OUTPUTS, 1024], f32)
                        for c in range(4):
                            nc.tensor.matmul(
                                po[:, c * 256:(c + 1) * 256],
                                wneg2_sb[:], vs[:, c * 256:(c + 1) * 256],
                                start=True, stop=True)
                        # drain + bias (per-partition scalar) fused
                        sslice = stg[:, g8 * 1024:(g8 + 1) * 1024]
                        if g8 == 0:
                            nc.vector.tensor_scalar(
                                sslice, po[:], bias_sb[:, 0:1], None, add)
                        else:
                            nc.scalar.activation(
                                sslice, po[:],
                                mybir.ActivationFunctionType.Identity,
                                bias=bias_sb[:, 0:1], scale=1.0)
                    nc.sync.dma_start(
                        outr[st, :, big * 16:(big + 1) * 16, :], stg[:])

    nc.compile()
    return nc


_CACHE = {}


def _get_compiled(gate_weights, output_weights, output_scale):
    key = hashlib.sha256(
        np.asarray(gate_weights, np.float32).tobytes()
        + np.asarray(output_weights, np.float32).tobytes()
        + np.asarray(output_scale, np.float32).tobytes()
    ).hexdigest()
    if key not in _CACHE:
        plan = _build_plan(gate_weights, output_weights, output_scale)
        nc = _build_bass_kernel(plan)
        _CACHE[key] = (plan, nc)
    return _CACHE[key]


def _decode_out(dev_out, plan, n_loc=N_LOC, nsub=NSUB):
    """[16, n_loc//2] transposed device output (bias included) ->
    [n_loc, 8]."""
    n_st = n_loc // (128 * nsub)
    o5 = np.asarray(dev_out).reshape(2, N_OUTPUTS, n_st, nsub // 2, 128)
    # [jj, o, st, pr, p] -> [st, p, pr, jj, o]
    return np.transpose(o5, (2, 4, 3, 0, 1)).reshape(n_loc, N_OUTPUTS)


def make_in_maps(X, plan, n_loc=N_LOC, nsub=NSUB, n_cores=N_CORES):
    bias16 = np.concatenate([plan["bias_f32"], plan["bias_f32"]])
    bias16 = bias16.reshape(2 * N_OUTPUTS, 1).astype(np.float32)
    n_st = n_loc // (128 * nsub)
    # slot-major bf16: xg[core][st, p, c, j] = X[...]
    xg = (X[:n_cores * n_loc]
          .reshape(n_cores, n_st, 128, nsub, N_FEATURES)
          .transpose(0, 1, 2, 4, 3)
          .astype(ml_dtypes.bfloat16))
    in_maps = []
    for c in range(n_cores):
        in_maps.append({
            "xg": np.ascontiguousarray(xg[c]),
            "wneg2": plan["wneg2_bf16"],
            "bias16": bias16,
        })
    return in_maps


def kernel(X, gate_weights, output_weights, output_scale):
    X = np.asarray(X, dtype=np.float32)
    plan, nc = _get_compiled(gate_weights, output_weights, output_scale)
    in_maps = make_in_maps(X, plan)

    from concourse.bass_utils import run_bass_kernel_spmd
    res = run_bass_kernel_spmd(nc, in_maps, list(range(N_CORES)))
    out = np.concatenate(
        [_decode_out(res.results[c]["out"], plan) for c in range(N_CORES)],
        axis=0)
    return out.astype(np.float32)


# revision 43
# speedup vs baseline: 1.9067x; 1.0571x over previous
"""Trainium2 Bass kernel for nn_CircuitBuilder (topk_masking).

Computes, for X [524288, 64] (f32), gate_weights [64, 130], output_weights
[64, 8], output_scale [8]:

    buf = [X | 0 | 1 | gate slots]
    top2[i] = top-2 of softmax(gate_weights[i, :66+i])   (data-independent
              of X; softmax is monotonic so = top-2 of masked logits)
    g_i = 1 - a*b  (continuous NAND chain, a/b gathered from buf)
    out = (gate_matrix @ output_weights) * output_scale

Strategy (pure data parallel over 8 NeuronCores, 65536 samples each):
  - The gate wiring is computed on host from gate_weights (tiny); the
    device kernel is built for that wiring.
  - Per-core layout: partition p owns 512 consecutive samples, processed
    as 2 supertiles of nsub=256 samples per partition. X tile is
    [128, nsub, 64] f32 (contiguous DMA); gate values live in a
    [128, 64, nsub] bf16 tile ("v" storage), where slot i holds
    v_i = alpha_i * (a_i*b_i) with a per-gate sign alpha chosen so each
    gate needs 1 fused DVE op (2 for gate×gate gates):
        m_i = a*b,  g_i = 1 - m_i,  v_i = alpha_i * m_i
    using scalar_tensor_tensor  out = (in0 op0 scalar) op1 in1.
  - Output: out = bias - sum_i W_i*m_i = bias + sum_i Wneg_i*v_i with
    Wneg_i = -alpha_i*W_i*scale. The v tile is rotated per 128-sample
    subtile with TensorE transposes into PSUM (packed bf16), drained to
    SBUF, then matmul'd against Wneg into [128, 8] psum chunks, bias
    added and DMA'd out.
"""

import hashlib
import sys
import types

import numpy as np
import ml_dtypes

N_SAMPLES = 524288
N_FEATURES = 64
N_GATES = 64
N_OUTPUTS = 8
BASE = N_FEATURES + 2            # 66
MAX_CONN = BASE + N_GATES        # 130
N_CORES = 8
N_LOC = N_SAMPLES // N_CORES     # 65536 samples per core
NSUB = 256                       # samples per partition per supertile
N_ST = N_LOC // (128 * NSUB)     # supertiles per core (2)


def _top2(gate_weights: np.ndarray) -> np.ndarray:
    """Top-2 connection indices per gate (matches jax.lax.top_k of the
    softmax: softmax is monotonic, top_k ties break to lower index,
    stable argsort of the negated row reproduces that)."""
    top2 = np.zeros((N_GATES, 2), dtype=np.int64)
    for i in range(N_GATES):
        row = np.asarray(gate_weights[i], dtype=np.float32).copy()
        row[BASE + i:] = -1e9
        top2[i] = np.argsort(-row, kind="stable")[:2]
    return top2


def _build_plan(gate_weights, output_weights, output_scale):
    """Host-side gate wiring -> per-gate op descriptors + output weights."""
    top2 = _top2(gate_weights)
    W = np.asarray(output_weights, dtype=np.float64)
    scale = np.asarray(output_scale, dtype=np.float64)

    ops = []          # list of dicts describing device ops per gate
    alpha = np.zeros(N_GATES, dtype=np.int64)
    for i in range(N_GATES):
        c0, c1 = int(top2[i][0]), int(top2[i][1])

        def kind(c):
            if c < N_FEATURES:
                return "x"
            if c == N_FEATURES:
                return "0"
            if c == N_FEATURES + 1:
                return "1"
            return "g"

        k0, k1 = kind(c0), kind(c1)
        # order canonically: g-operands first, then x, consts last
        pri = {"g": 0, "x": 1, "1": 2, "0": 3}
        if pri[k0] > pri[k1]:
            c0, c1, k0, k1 = c1, c0, k1, k0
        kk = k0 + k1
        if "0" in kk:
            ops.append({"op": "zero", "i": i})
            alpha[i] = 1
        elif kk == "x1":
            ops.append({"op": "copyx", "i": i, "c": c0})
            alpha[i] = 1
        elif kk == "g1":
            p = c0 - BASE
            ops.append({"op": "g1", "i": i, "p": p, "ap": int(alpha[p])})
            alpha[i] = 1
        elif kk == "xx":
            ops.append({"op": "xx", "i": i, "a": c0, "b": c1})
            alpha[i] = 1
        elif kk == "gx":
            p = c0 - BASE
            ap = int(alpha[p])
            ops.append({"op": "gx", "i": i, "p": p, "c": c1, "ap": ap})
            alpha[i] = -ap
        elif kk == "gg":
            p, q = c0 - BASE, c1 - BASE
            ap, aq = int(alpha[p]), int(alpha[q])
            if aq != -1 and ap == -1:
                p, q, ap, aq = q, p, aq, ap
            ops.append({"op": "gg", "i": i, "p": p, "q": q, "ap": ap, "aq": aq})
            alpha[i] = -ap if (aq == -1) else 1
        else:
            raise AssertionError(f"unexpected operand kinds {kk}")

    wneg = (-alpha[:, None] * W * scale[None, :]).astype(np.float64)
    bias = (W * scale[None, :]).sum(axis=0)
    # block-diagonal-by-subtile-parity projection matrix for the
    # pair-transpose scheme: row (g*2 + jj), col (jj*8 + o) = wneg[g, o]
    wneg2 = np.zeros((2 * N_GATES, 2 * N_OUTPUTS), dtype=np.float64)
    for jj in range(2):
        wneg2[jj::2, jj * N_OUTPUTS:(jj + 1) * N_OUTPUTS] = wneg
    return {
        "ops": ops,
        "alpha": alpha,
        "wneg_bf16": wneg.astype(ml_dtypes.bfloat16),
        "wneg2_bf16": wneg2.astype(ml_dtypes.bfloat16),
        "bias_f32": bias.astype(np.float32),
        "top2": top2,
    }


def _emulate_plan(plan, X):
    """Numpy emulation of the device program (bf16 v-storage) — used for
    host-side self-checks in development."""
    n = X.shape[0]
    bf = ml_dtypes.bfloat16
    V = np.zeros((N_GATES, n), dtype=bf)
    # device receives X pre-quantized to bf16
    Xc = np.asarray(X, dtype=np.float32).T.astype(bf).astype(np.float32)
    for d in plan["ops"]:
        i = d["i"]
        if d["op"] == "zero":
            V[i] = 0
        elif d["op"] == "copyx":
            V[i] = Xc[d["c"]].astype(bf)
        elif d["op"] == "g1":
            V[i] = (V[d["p"]].astype(np.float32) * (-d["ap"]) + 1.0).astype(bf)
        elif d["op"] == "xx":
            V[i] = (Xc[d["a"]] * Xc[d["b"]]).astype(bf)
        elif d["op"] == "gx":
            cp = -d["ap"]
            V[i] = ((V[d["p"]].astype(np.float32) + cp) * Xc[d["c"]]).astype(bf)
        elif d["op"] == "gg":
            vp = V[d["p"]].astype(np.float32)
            vq = V[d["q"]].astype(np.float32)
            if d["aq"] == -1 or d["ap"] == -1:
                cp = -d["ap"]
                t = ((vp + cp) * vq).astype(bf).astype(np.float32)
                V[i] = ((vp + cp) + t).astype(bf)
            else:
                t = ((vp - 1.0) * vq).astype(bf).astype(np.float32)
                V[i] = ((t + 1.0) - vp).astype(bf)
    wneg = plan["wneg_bf16"].astype(np.float32)
    out = V.astype(np.float32).T @ wneg + plan["bias_f32"][None, :]
    return out


def _build_bass_kernel(plan, n_loc=N_LOC, nsub=NSUB, sim_safe=False):
    import concourse.bacc as bacc
    import concourse.tile as tile
    import concourse.mybir as mybir
    from concourse import masks

    f32 = mybir.dt.float32
    bf16 = mybir.dt.bfloat16
    mult = mybir.AluOpType.mult
    add = mybir.AluOpType.add
    subtract = mybir.AluOpType.subtract

    n_st = n_loc // (128 * nsub)
    assert n_st * 128 * nsub == n_loc

    nc = bacc.Bacc(None, target_bir_lowering=False)
    # x pre-transposed on host to slot-major bf16:
    # xg[st, p, c, j] = X[st*128*nsub + p*nsub + j, c]
    x_d = nc.dram_tensor("xg", [n_st, 128, N_FEATURES, nsub], bf16,
                         kind="ExternalInput")
    wneg2_d = nc.dram_tensor("wneg2", [2 * N_GATES, 2 * N_OUTPUTS], bf16,
                             kind="ExternalInput")
    bias_d = nc.dram_tensor("bias48", [48, 1], f32, kind="ExternalInput")
    # transposed output: row (jj*8 + o), col = (st, pair, p); host decodes
    out_d = nc.dram_tensor("out", [2 * N_OUTPUTS, n_loc // 2], f32,
                           kind="ExternalOutput")

    outr = out_d.rearrange("r (s q pp) -> s r q pp",
                           s=n_st, q=nsub // 2, pp=128)

    with tile.TileContext(nc) as tc:
        with (
            tc.tile_pool(name="const", bufs=1) as cpool,
            tc.tile_pool(name="xp", bufs=2) as xpool,
            tc.tile_pool(name="vp", bufs=2) as vpool,
            tc.tile_pool(name="tp", bufs=4) as tpool,
            tc.tile_pool(name="vs", bufs=4) as vspool,
            tc.tile_pool(name="stg", bufs=2) as stgpool,
            tc.tile_pool(name="pt", bufs=4, space="PSUM") as ptpool,
            tc.tile_pool(name="po", bufs=2, space="PSUM") as popool,
        ):
            ident = cpool.tile([128, 128], bf16)
            masks.make_identity(nc, ident[:])
            wneg2_sb = cpool.tile([2 * N_GATES, 2 * N_OUTPUTS], bf16)
            nc.sync.dma_start(wneg2_sb[:], wneg2_d[:])
            bias_sb = cpool.tile([48, 1], f32)
            nc.sync.dma_start(bias_sb[:], bias_d[:])

            for st in range(n_st):
                # x: [p, col, pair, jj] slot-major bf16 (host-transposed);
                # v: [p, pair, (gate*2 + jj)] so each pair-block is
                # contiguous (1-free-dim transpose input)
                xt = xpool.tile([128, N_FEATURES, nsub // 2, 2], bf16)
                for k in range(2):
                    nc.sync.dma_start(
                        xt[:, k * 32:(k + 1) * 32, :, :],
                        x_d[st, :, k * 32:(k + 1) * 32, :].rearrange(
                            "p c (q t) -> p c q t", t=2))
                vt = vpool.tile([128, nsub // 2, 2 * N_GATES], bf16)

                def xcol(c):
                    return xt[:, c, :, :]

                def vslot(i):
                    return vt[:, :, 2 * i:2 * i + 2]

                for d in plan["ops"]:
                    i = d["i"]
                    o = d["op"]
                    if o == "zero":
                        nc.vector.memset(vslot(i), 0.0)
                    elif o == "copyx":
                        nc.vector.tensor_copy(vslot(i), xcol(d["c"]))
                    elif o == "g1":
                        nc.scalar.activation(
                            vslot(i), vslot(d["p"]),
                            mybir.ActivationFunctionType.Identity,
                            bias=1.0, scale=float(-d["ap"]))
                    elif o == "xx":
                        # stt form measures faster than plain TENSOR_TENSOR
                        nc.vector.scalar_tensor_tensor(
                            vslot(i), xcol(d["a"]), 1.0,
                            xcol(d["b"]), mult, mult)
                    elif o == "gx":
                        nc.vector.scalar_tensor_tensor(
                            vslot(i), vslot(d["p"]), float(-d["ap"]),
                            xcol(d["c"]), add, mult)
                    elif o == "gg":
                        t = tpool.tile([128, nsub // 2, 2], bf16, tag="ggtmp")
                        if d["aq"] == -1 or d["ap"] == -1:
                            cp = float(-d["ap"])
                            nc.vector.scalar_tensor_tensor(
                                t[:], vslot(d["p"]), cp, vslot(d["q"]),
                                add, mult)
                            nc.vector.scalar_tensor_tensor(
                                vslot(i), vslot(d["p"]), cp, t[:],
                                add, add)
                        else:
                            nc.vector.scalar_tensor_tensor(
                                t[:], vslot(d["p"]), -1.0, vslot(d["q"]),
                                add, mult)
                            nc.vector.scalar_tensor_tensor(
                                vslot(i), t[:], 1.0, vslot(d["p"]),
                                add, subtract)

                # output: transpose subtile PAIRS ([128, 64g x 2j] input ->
                # [128 rows=(g,jj), 128 samples]) and project with the
                # block-diagonal wneg2 (constant stationary).  PE lhsT/rhs
                # stay at base partition 0 (base-64 operands flap the PE
                # tile config and crash at scale); projection OUTPUTS pack
                # two groups per psum tile at partition offsets {0, 32} so
                # one fused bias+drain covers 16 pairs.  Host
                # de-interleaves the [16, n] transposed output.
                for big in range(nsub // 32):     # 16 pairs per iteration
                    stg = stgpool.tile([48, 1024], f32)
                    po = popool.tile([48, 1024], f32)
                    for g8 in range(2):           # 8 pairs per pt bank
                        pt = ptpool.tile([128, 1024], bf16)
                        for c in range(8):
                            pr = big * 16 + g8 * 8 + c
                            nc.tensor.transpose(
                                pt[:, c * 128:(c + 1) * 128],
                                vt[:, pr, :], ident[:])
                        vs = vspool.tile([128, 1024], bf16)
                        if g8 == 0:
                            nc.scalar.copy(vs[:], pt[:])
                        else:
                            nc.vector.tensor_copy(vs[:], pt[:])
                        for c in range(2):
                            nc.tensor.matmul(
                                po[32 * g8:32 * g8 + 16,
                                   c * 512:(c + 1) * 512],
                                wneg2_sb[:], vs[:, c * 512:(c + 1) * 512],
                                start=True, stop=True)
                    # drain + bias (per-partition scalar) fused, 16 pairs.
                    # The single [48, ...] op reads the unwritten psum gap
                    # rows 16-31 (never DMA'd, benign on HW); CoreSim
                    # flags uninitialized reads, so sim builds drain the
                    # two written slices instead.
                    drains = ([(stg[:], po[:], bias_sb[:, 0:1])]
                              if not sim_safe else
                              [(stg[0:16, :], po[0:16, :], bias_sb[0:16, 0:1]),
                               (stg[32:48, :], po[32:48, :],
                                bias_sb[32:48, 0:1])])
                    for sslice, pslice, bslice in drains:
                        if big % 2 == 0:
                            nc.vector.tensor_scalar(
                                sslice, pslice, bslice, None, add)
                        else:
                            nc.scalar.activation(
                                sslice, pslice,
                                mybir.ActivationFunctionType.Identity,
                                bias=bslice, scale=1.0)
                    for g8 in range(2):
                        nc.sync.dma_start(
                            outr[st, :,
                                 big * 16 + g8 * 8:big * 16 + g8 * 8 + 8, :],
                            stg[32 * g8:32 * g8 + 16, :])

    nc.compile()
    return nc


_CACHE = {}


def _get_compiled(gate_weights, output_weights, output_scale):
    key = hashlib.sha256(
        np.asarray(gate_weights, np.float32).tobytes()
        + np.asarray(output_weights, np.float32).tobytes()
        + np.asarray(output_scale, np.float32).tobytes()
    ).hexdigest()
    if key not in _CACHE:
        plan = _build_plan(gate_weights, output_weights, output_scale)
        nc = _build_bass_kernel(plan)
        _CACHE[key] = (plan, nc)
    return _CACHE[key]


def _decode_out(dev_out, plan, n_loc=N_LOC, nsub=NSUB):
    """[16, n_loc//2] transposed device output (bias included) ->
    [n_loc, 8]."""
    n_st = n_loc // (128 * nsub)
    o5 = np.asarray(dev_out).reshape(2, N_OUTPUTS, n_st, nsub // 2, 128)
    # [jj, o, st, pr, p] -> [st, p, pr, jj, o]
    return np.transpose(o5, (2, 4, 3, 0, 1)).reshape(n_loc, N_OUTPUTS)


def make_in_maps(X, plan, n_loc=N_LOC, nsub=NSUB, n_cores=N_CORES):
    bias16 = np.concatenate([plan["bias_f32"], plan["bias_f32"]])
    bias48 = np.zeros((48, 1), dtype=np.float32)
    bias48[0:16, 0] = bias16
    bias48[32:48, 0] = bias16
    n_st = n_loc // (128 * nsub)
    # slot-major bf16: xg[core][st, p, c, j] = X[...]
    xg = (X[:n_cores * n_loc]
          .reshape(n_cores, n_st, 128, nsub, N_FEATURES)
          .transpose(0, 1, 2, 4, 3)
          .astype(ml_dtypes.bfloat16))
    in_maps = []
    for c in range(n_cores):
        in_maps.append({
            "xg": np.ascontiguousarray(xg[c]),
            "wneg2": plan["wneg2_bf16"],
            "bias48": bias48,
        })
    return in_maps


def kernel(X, gate_weights, output_weights, output_scale):
    X = np.asarray(X, dtype=np.float32)
    plan, nc = _get_compiled(gate_weights, output_weights, output_scale)
    in_maps = make_in_maps(X, plan)

    from concourse.bass_utils import run_bass_kernel_spmd
    res = run_bass_kernel_spmd(nc, in_maps, list(range(N_CORES)))
    out = np.concatenate(
        [_decode_out(res.results[c]["out"], plan) for c in range(N_CORES)],
        axis=0)
    return out.astype(np.float32)


# revision 46
# speedup vs baseline: 2.0221x; 1.0605x over previous
"""Trainium2 Bass kernel for nn_CircuitBuilder (topk_masking).

Computes, for X [524288, 64] (f32), gate_weights [64, 130], output_weights
[64, 8], output_scale [8]:

    buf = [X | 0 | 1 | gate slots]
    top2[i] = top-2 of softmax(gate_weights[i, :66+i])   (data-independent
              of X; softmax is monotonic so = top-2 of masked logits)
    g_i = 1 - a*b  (continuous NAND chain, a/b gathered from buf)
    out = (gate_matrix @ output_weights) * output_scale

Strategy (pure data parallel over 8 NeuronCores, 65536 samples each):
  - The gate wiring is computed on host from gate_weights (tiny); the
    device kernel is built for that wiring.
  - Per-core layout: partition p owns 512 consecutive samples, processed
    as 2 supertiles of nsub=256 samples per partition. X tile is
    [128, nsub, 64] f32 (contiguous DMA); gate values live in a
    [128, 64, nsub] bf16 tile ("v" storage), where slot i holds
    v_i = alpha_i * (a_i*b_i) with a per-gate sign alpha chosen so each
    gate needs 1 fused DVE op (2 for gate×gate gates):
        m_i = a*b,  g_i = 1 - m_i,  v_i = alpha_i * m_i
    using scalar_tensor_tensor  out = (in0 op0 scalar) op1 in1.
  - Output: out = bias - sum_i W_i*m_i = bias + sum_i Wneg_i*v_i with
    Wneg_i = -alpha_i*W_i*scale. The v tile is rotated per 128-sample
    subtile with TensorE transposes into PSUM (packed bf16), drained to
    SBUF, then matmul'd against Wneg into [128, 8] psum chunks, bias
    added and DMA'd out.
"""

import hashlib
import sys
import types

import numpy as np
import ml_dtypes

N_SAMPLES = 524288
N_FEATURES = 64
N_GATES = 64
N_OUTPUTS = 8
BASE = N_FEATURES + 2            # 66
MAX_CONN = BASE + N_GATES        # 130
N_CORES = 8
N_LOC = N_SAMPLES // N_CORES     # 65536 samples per core
NSUB = 256                       # samples per partition per supertile
N_ST = N_LOC // (128 * NSUB)     # supertiles per core (2)


def _top2(gate_weights: np.ndarray) -> np.ndarray:
    """Top-2 connection indices per gate (matches jax.lax.top_k of the
    softmax: softmax is monotonic, top_k ties break to lower index,
    stable argsort of the negated row reproduces that)."""
    top2 = np.zeros((N_GATES, 2), dtype=np.int64)
    for i in range(N_GATES):
        row = np.asarray(gate_weights[i], dtype=np.float32).copy()
        row[BASE + i:] = -1e9
        top2[i] = np.argsort(-row, kind="stable")[:2]
    return top2


def _build_plan(gate_weights, output_weights, output_scale):
    """Host-side gate wiring -> per-gate op descriptors + output weights."""
    top2 = _top2(gate_weights)
    W = np.asarray(output_weights, dtype=np.float64)
    scale = np.asarray(output_scale, dtype=np.float64)

    ops = []          # list of dicts describing device ops per gate
    alpha = np.zeros(N_GATES, dtype=np.int64)
    for i in range(N_GATES):
        c0, c1 = int(top2[i][0]), int(top2[i][1])

        def kind(c):
            if c < N_FEATURES:
                return "x"
            if c == N_FEATURES:
                return "0"
            if c == N_FEATURES + 1:
                return "1"
            return "g"

        k0, k1 = kind(c0), kind(c1)
        # order canonically: g-operands first, then x, consts last
        pri = {"g": 0, "x": 1, "1": 2, "0": 3}
        if pri[k0] > pri[k1]:
            c0, c1, k0, k1 = c1, c0, k1, k0
        kk = k0 + k1
        if "0" in kk:
            ops.append({"op": "zero", "i": i})
            alpha[i] = 1
        elif kk == "x1":
            ops.append({"op": "copyx", "i": i, "c": c0})
            alpha[i] = 1
        elif kk == "g1":
            p = c0 - BASE
            ops.append({"op": "g1", "i": i, "p": p, "ap": int(alpha[p])})
            alpha[i] = 1
        elif kk == "xx":
            ops.append({"op": "xx", "i": i, "a": c0, "b": c1})
            alpha[i] = 1
        elif kk == "gx":
            p = c0 - BASE
            ap = int(alpha[p])
            ops.append({"op": "gx", "i": i, "p": p, "c": c1, "ap": ap})
            alpha[i] = -ap
        elif kk == "gg":
            p, q = c0 - BASE, c1 - BASE
            ap, aq = int(alpha[p]), int(alpha[q])
            if aq != -1 and ap == -1:
                p, q, ap, aq = q, p, aq, ap
            ops.append({"op": "gg", "i": i, "p": p, "q": q, "ap": ap, "aq": aq})
            alpha[i] = -ap if (aq == -1) else 1
        else:
            raise AssertionError(f"unexpected operand kinds {kk}")

    wneg = (-alpha[:, None] * W * scale[None, :]).astype(np.float64)
    bias = (W * scale[None, :]).sum(axis=0)
    # block-diagonal-by-subtile-parity projection matrix for the
    # pair-transpose scheme: row (g*2 + jj), col (jj*8 + o) = wneg[g, o]
    wneg2 = np.zeros((2 * N_GATES, 2 * N_OUTPUTS), dtype=np.float64)
    for jj in range(2):
        wneg2[jj::2, jj * N_OUTPUTS:(jj + 1) * N_OUTPUTS] = wneg
    return {
        "ops": ops,
        "alpha": alpha,
        "wneg_bf16": wneg.astype(ml_dtypes.bfloat16),
        "wneg2_bf16": wneg2.astype(ml_dtypes.bfloat16),
        "bias_f32": bias.astype(np.float32),
        "top2": top2,
    }


def _emulate_plan(plan, X):
    """Numpy emulation of the device program (bf16 v-storage) — used for
    host-side self-checks in development."""
    n = X.shape[0]
    bf = ml_dtypes.bfloat16
    V = np.zeros((N_GATES, n), dtype=bf)
    # device receives X pre-quantized to bf16
    Xc = np.asarray(X, dtype=np.float32).T.astype(bf).astype(np.float32)
    for d in plan["ops"]:
        i = d["i"]
        if d["op"] == "zero":
            V[i] = 0
        elif d["op"] == "copyx":
            V[i] = Xc[d["c"]].astype(bf)
        elif d["op"] == "g1":
            V[i] = (V[d["p"]].astype(np.float32) * (-d["ap"]) + 1.0).astype(bf)
        elif d["op"] == "xx":
            V[i] = (Xc[d["a"]] * Xc[d["b"]]).astype(bf)
        elif d["op"] == "gx":
            cp = -d["ap"]
            V[i] = ((V[d["p"]].astype(np.float32) + cp) * Xc[d["c"]]).astype(bf)
        elif d["op"] == "gg":
            vp = V[d["p"]].astype(np.float32)
            vq = V[d["q"]].astype(np.float32)
            if d["aq"] == -1 or d["ap"] == -1:
                cp = -d["ap"]
                t = ((vp + cp) * vq).astype(bf).astype(np.float32)
                V[i] = ((vp + cp) + t).astype(bf)
            else:
                t = ((vp - 1.0) * vq).astype(bf).astype(np.float32)
                V[i] = ((t + 1.0) - vp).astype(bf)
    wneg = plan["wneg_bf16"].astype(np.float32)
    out = V.astype(np.float32).T @ wneg + plan["bias_f32"][None, :]
    return out


def _build_bass_kernel(plan, n_loc=N_LOC, nsub=NSUB, sim_safe=False):
    import concourse.bacc as bacc
    import concourse.tile as tile
    import concourse.mybir as mybir
    from concourse import masks

    f32 = mybir.dt.float32
    bf16 = mybir.dt.bfloat16
    mult = mybir.AluOpType.mult
    add = mybir.AluOpType.add
    subtract = mybir.AluOpType.subtract

    n_st = n_loc // (128 * nsub)
    assert n_st * 128 * nsub == n_loc

    nc = bacc.Bacc(None, target_bir_lowering=False)
    # x pre-transposed on host to slot-major bf16:
    # xg[st, p, c, j] = X[st*128*nsub + p*nsub + j, c]
    x_d = nc.dram_tensor("xg", [n_st, 128, N_FEATURES, nsub], bf16,
                         kind="ExternalInput")
    wneg2_d = nc.dram_tensor("wneg2", [2 * N_GATES, 2 * N_OUTPUTS], bf16,
                             kind="ExternalInput")
    bias_d = nc.dram_tensor("bias48", [48, 1], f32, kind="ExternalInput")
    # transposed output: row (jj*8 + o), col = (st, pair, p); host decodes
    out_d = nc.dram_tensor("out", [2 * N_OUTPUTS, n_loc // 2], f32,
                           kind="ExternalOutput")

    outr = out_d.rearrange("r (s q pp) -> s r q pp",
                           s=n_st, q=nsub // 2, pp=128)

    with tile.TileContext(nc) as tc:
        with (
            tc.tile_pool(name="const", bufs=1) as cpool,
            tc.tile_pool(name="xp", bufs=2) as xpool,
            tc.tile_pool(name="vp", bufs=2) as vpool,
            tc.tile_pool(name="tp", bufs=4) as tpool,
            tc.tile_pool(name="vs", bufs=4) as vspool,
            tc.tile_pool(name="stg", bufs=2) as stgpool,
            tc.tile_pool(name="pt", bufs=4, space="PSUM") as ptpool,
            tc.tile_pool(name="po", bufs=2, space="PSUM") as popool,
        ):
            ident = cpool.tile([128, 128], bf16)
            masks.make_identity(nc, ident[:])
            wneg2_sb = cpool.tile([2 * N_GATES, 2 * N_OUTPUTS], bf16)
            nc.sync.dma_start(wneg2_sb[:], wneg2_d[:])
            bias_sb = cpool.tile([48, 1], f32)
            nc.sync.dma_start(bias_sb[:], bias_d[:])

            for st in range(n_st):
                # x: [p, col, pair, jj] slot-major bf16 (host-transposed);
                # v: [p, pair, (gate*2 + jj)] so each pair-block is
                # contiguous (1-free-dim transpose input)
                xt = xpool.tile([128, N_FEATURES, nsub // 2, 2], bf16)
                for k in range(2):
                    nc.sync.dma_start(
                        xt[:, k * 32:(k + 1) * 32, :, :],
                        x_d[st, :, k * 32:(k + 1) * 32, :].rearrange(
                            "p c (q t) -> p c q t", t=2))
                vt = vpool.tile([128, nsub // 2, 2 * N_GATES], bf16)

                def xcol(c):
                    return xt[:, c, :, :]

                def vslot(i):
                    return vt[:, :, 2 * i:2 * i + 2]

                for d in plan["ops"]:
                    i = d["i"]
                    o = d["op"]
                    if o == "zero":
                        nc.vector.memset(vslot(i), 0.0)
                    elif o == "copyx":
                        nc.vector.tensor_copy(vslot(i), xcol(d["c"]))
                    elif o == "g1":
                        nc.scalar.activation(
                            vslot(i), vslot(d["p"]),
                            mybir.ActivationFunctionType.Identity,
                            bias=1.0, scale=float(-d["ap"]))
                    elif o == "xx":
                        # stt form measures faster than plain TENSOR_TENSOR
                        nc.vector.scalar_tensor_tensor(
                            vslot(i), xcol(d["a"]), 1.0,
                            xcol(d["b"]), mult, mult)
                    elif o == "gx":
                        nc.vector.scalar_tensor_tensor(
                            vslot(i), vslot(d["p"]), float(-d["ap"]),
                            xcol(d["c"]), add, mult)
                    elif o == "gg":
                        t = tpool.tile([128, nsub // 2, 2], bf16, tag="ggtmp")
                        if d["aq"] == -1 or d["ap"] == -1:
                            cp = float(-d["ap"])
                            nc.vector.scalar_tensor_tensor(
                                t[:], vslot(d["p"]), cp, vslot(d["q"]),
                                add, mult)
                            nc.vector.scalar_tensor_tensor(
                                vslot(i), vslot(d["p"]), cp, t[:],
                                add, add)
                        else:
                            nc.vector.scalar_tensor_tensor(
                                t[:], vslot(d["p"]), -1.0, vslot(d["q"]),
                                add, mult)
                            nc.vector.scalar_tensor_tensor(
                                vslot(i), t[:], 1.0, vslot(d["p"]),
                                add, subtract)

                # output: transpose subtile PAIRS ([128, 64g x 2j] input ->
                # [128 rows=(g,jj), 128 samples]) and project with the
                # block-diagonal wneg2 (constant stationary).  PE lhsT/rhs
                # stay at base partition 0 (base-64 operands flap the PE
                # tile config and crash at scale); projection OUTPUTS pack
                # two groups per psum tile at partition offsets {0, 32} so
                # one fused bias+drain covers 16 pairs.  Host
                # de-interleaves the [16, n] transposed output.
                for big in range(nsub // 32):     # 16 pairs per iteration
                    stg = stgpool.tile([48, 1024], f32)
                    po = popool.tile([48, 1024], f32)
                    for g8 in range(2):           # 8 pairs per pt bank
                        pt = ptpool.tile([128, 1024], bf16)
                        for c in range(8):
                            pr = big * 16 + g8 * 8 + c
                            nc.tensor.transpose(
                                pt[:, c * 128:(c + 1) * 128],
                                vt[:, pr, :], ident[:])
                        vs = vspool.tile([128, 1024], bf16)
                        nc.scalar.copy(vs[:], pt[:])
                        for c in range(2):
                            nc.tensor.matmul(
                                po[32 * g8:32 * g8 + 16,
                                   c * 512:(c + 1) * 512],
                                wneg2_sb[:], vs[:, c * 512:(c + 1) * 512],
                                start=True, stop=True)
                    # drain + bias (per-partition scalar) fused, 16 pairs.
                    # The single [48, ...] op reads the unwritten psum gap
                    # rows 16-31 (never DMA'd, benign on HW); CoreSim
                    # flags uninitialized reads, so sim builds drain the
                    # two written slices instead.
                    drains = ([(stg[:], po[:], bias_sb[:, 0:1])]
                              if not sim_safe else
                              [(stg[0:16, :], po[0:16, :], bias_sb[0:16, 0:1]),
                               (stg[32:48, :], po[32:48, :],
                                bias_sb[32:48, 0:1])])
                    for sslice, pslice, bslice in drains:
                        nc.scalar.activation(
                            sslice, pslice,
                            mybir.ActivationFunctionType.Identity,
                            bias=bslice, scale=1.0)
                    for g8 in range(2):
                        nc.sync.dma_start(
                            outr[st, :,
                                 big * 16 + g8 * 8:big * 16 + g8 * 8 + 8, :],
                            stg[32 * g8:32 * g8 + 16, :])

    nc.compile()
    return nc


_CACHE = {}


def _get_compiled(gate_weights, output_weights, output_scale):
    key = hashlib.sha256(
        np.asarray(gate_weights, np.float32).tobytes()
        + np.asarray(output_weights, np.float32).tobytes()
        + np.asarray(output_scale, np.float32).tobytes()
    ).hexdigest()
    if key not in _CACHE:
        plan = _build_plan(gate_weights, output_weights, output_scale)
        nc = _build_bass_kernel(plan)
        _CACHE[key] = (plan, nc)
    return _CACHE[key]


def _decode_out(dev_out, plan, n_loc=N_LOC, nsub=NSUB):
    """[16, n_loc//2] transposed device output (bias included) ->
    [n_loc, 8]."""
    n_st = n_loc // (128 * nsub)
    o5 = np.asarray(dev_out).reshape(2, N_OUTPUTS, n_st, nsub // 2, 128)
    # [jj, o, st, pr, p] -> [st, p, pr, jj, o]
    return np.transpose(o5, (2, 4, 3, 0, 1)).reshape(n_loc, N_OUTPUTS)


def make_in_maps(X, plan, n_loc=N_LOC, nsub=NSUB, n_cores=N_CORES):
    bias16 = np.concatenate([plan["bias_f32"], plan["bias_f32"]])
    bias48 = np.zeros((48, 1), dtype=np.float32)
    bias48[0:16, 0] = bias16
    bias48[32:48, 0] = bias16
    n_st = n_loc // (128 * nsub)
    # slot-major bf16: xg[core][st, p, c, j] = X[...]
    xg = (X[:n_cores * n_loc]
          .reshape(n_cores, n_st, 128, nsub, N_FEATURES)
          .transpose(0, 1, 2, 4, 3)
          .astype(ml_dtypes.bfloat16))
    in_maps = []
    for c in range(n_cores):
        in_maps.append({
            "xg": np.ascontiguousarray(xg[c]),
            "wneg2": plan["wneg2_bf16"],
            "bias48": bias48,
        })
    return in_maps


def kernel(X, gate_weights, output_weights, output_scale):
    X = np.asarray(X, dtype=np.float32)
    plan, nc = _get_compiled(gate_weights, output_weights, output_scale)
    in_maps = make_in_maps(X, plan)

    from concourse.bass_utils import run_bass_kernel_spmd
    res = run_bass_kernel_spmd(nc, in_maps, list(range(N_CORES)))
    out = np.concatenate(
        [_decode_out(res.results[c]["out"], plan) for c in range(N_CORES)],
        axis=0)
    return out.astype(np.float32)


# revision 48
# speedup vs baseline: 2.0572x; 1.0174x over previous
"""Trainium2 Bass kernel for nn_CircuitBuilder (topk_masking).

Computes, for X [524288, 64] (f32), gate_weights [64, 130], output_weights
[64, 8], output_scale [8]:

    buf = [X | 0 | 1 | gate slots]
    top2[i] = top-2 of softmax(gate_weights[i, :66+i])   (data-independent
              of X; softmax is monotonic so = top-2 of masked logits)
    g_i = 1 - a*b  (continuous NAND chain, a/b gathered from buf)
    out = (gate_matrix @ output_weights) * output_scale

Strategy (pure data parallel over 8 NeuronCores, 65536 samples each):
  - The gate wiring is computed on host from gate_weights (tiny); the
    device kernel is built for that wiring.
  - Per-core layout: partition p owns 512 consecutive samples, processed
    as 2 supertiles of nsub=256 samples per partition. X tile is
    [128, nsub, 64] f32 (contiguous DMA); gate values live in a
    [128, 64, nsub] bf16 tile ("v" storage), where slot i holds
    v_i = alpha_i * (a_i*b_i) with a per-gate sign alpha chosen so each
    gate needs 1 fused DVE op (2 for gate×gate gates):
        m_i = a*b,  g_i = 1 - m_i,  v_i = alpha_i * m_i
    using scalar_tensor_tensor  out = (in0 op0 scalar) op1 in1.
  - Output: out = bias - sum_i W_i*m_i = bias + sum_i Wneg_i*v_i with
    Wneg_i = -alpha_i*W_i*scale. The v tile is rotated per 128-sample
    subtile with TensorE transposes into PSUM (packed bf16), drained to
    SBUF, then matmul'd against Wneg into [128, 8] psum chunks, bias
    added and DMA'd out.
"""

import hashlib
import sys
import types

import numpy as np
import ml_dtypes

N_SAMPLES = 524288
N_FEATURES = 64
N_GATES = 64
N_OUTPUTS = 8
BASE = N_FEATURES + 2            # 66
MAX_CONN = BASE + N_GATES        # 130
N_CORES = 8
N_LOC = N_SAMPLES // N_CORES     # 65536 samples per core
NSUB = 256                       # samples per partition per supertile
N_ST = N_LOC // (128 * NSUB)     # supertiles per core (2)


def _top2(gate_weights: np.ndarray) -> np.ndarray:
    """Top-2 connection indices per gate (matches jax.lax.top_k of the
    softmax: softmax is monotonic, top_k ties break to lower index,
    stable argsort of the negated row reproduces that)."""
    top2 = np.zeros((N_GATES, 2), dtype=np.int64)
    for i in range(N_GATES):
        row = np.asarray(gate_weights[i], dtype=np.float32).copy()
        row[BASE + i:] = -1e9
        top2[i] = np.argsort(-row, kind="stable")[:2]
    return top2


def _build_plan(gate_weights, output_weights, output_scale):
    """Host-side gate wiring -> per-gate op descriptors + output weights."""
    top2 = _top2(gate_weights)
    W = np.asarray(output_weights, dtype=np.float64)
    scale = np.asarray(output_scale, dtype=np.float64)

    ops = []          # list of dicts describing device ops per gate
    alpha = np.zeros(N_GATES, dtype=np.int64)
    for i in range(N_GATES):
        c0, c1 = int(top2[i][0]), int(top2[i][1])

        def kind(c):
            if c < N_FEATURES:
                return "x"
            if c == N_FEATURES:
                return "0"
            if c == N_FEATURES + 1:
                return "1"
            return "g"

        k0, k1 = kind(c0), kind(c1)
        # order canonically: g-operands first, then x, consts last
        pri = {"g": 0, "x": 1, "1": 2, "0": 3}
        if pri[k0] > pri[k1]:
            c0, c1, k0, k1 = c1, c0, k1, k0
        kk = k0 + k1
        if "0" in kk:
            ops.append({"op": "zero", "i": i})
            alpha[i] = 1
        elif kk == "x1":
            ops.append({"op": "copyx", "i": i, "c": c0})
            alpha[i] = 1
        elif kk == "g1":
            p = c0 - BASE
            ops.append({"op": "g1", "i": i, "p": p, "ap": int(alpha[p])})
            alpha[i] = 1
        elif kk == "xx":
            ops.append({"op": "xx", "i": i, "a": c0, "b": c1})
            alpha[i] = 1
        elif kk == "gx":
            p = c0 - BASE
            ap = int(alpha[p])
            ops.append({"op": "gx", "i": i, "p": p, "c": c1, "ap": ap})
            alpha[i] = -ap
        elif kk == "gg":
            p, q = c0 - BASE, c1 - BASE
            ap, aq = int(alpha[p]), int(alpha[q])
            if aq != -1 and ap == -1:
                p, q, ap, aq = q, p, aq, ap
            ops.append({"op": "gg", "i": i, "p": p, "q": q, "ap": ap, "aq": aq})
            alpha[i] = -ap if (aq == -1) else 1
        else:
            raise AssertionError(f"unexpected operand kinds {kk}")

    wneg = (-alpha[:, None] * W * scale[None, :]).astype(np.float64)
    bias = (W * scale[None, :]).sum(axis=0)
    # block-diagonal-by-subtile-parity projection matrix for the
    # pair-transpose scheme: row (g*2 + jj), col (jj*8 + o) = wneg[g, o]
    wneg2 = np.zeros((2 * N_GATES, 2 * N_OUTPUTS), dtype=np.float64)
    for jj in range(2):
        wneg2[jj::2, jj * N_OUTPUTS:(jj + 1) * N_OUTPUTS] = wneg
    return {
        "ops": ops,
        "alpha": alpha,
        "wneg_bf16": wneg.astype(ml_dtypes.bfloat16),
        "wneg2_bf16": wneg2.astype(ml_dtypes.bfloat16),
        "bias_f32": bias.astype(np.float32),
        "top2": top2,
    }


def _emulate_plan(plan, X):
    """Numpy emulation of the device program (bf16 v-storage) — used for
    host-side self-checks in development."""
    n = X.shape[0]
    bf = ml_dtypes.bfloat16
    V = np.zeros((N_GATES, n), dtype=bf)
    # device receives X pre-quantized to bf16
    Xc = np.asarray(X, dtype=np.float32).T.astype(bf).astype(np.float32)
    for d in plan["ops"]:
        i = d["i"]
        if d["op"] == "zero":
            V[i] = 0
        elif d["op"] == "copyx":
            V[i] = Xc[d["c"]].astype(bf)
        elif d["op"] == "g1":
            V[i] = (V[d["p"]].astype(np.float32) * (-d["ap"]) + 1.0).astype(bf)
        elif d["op"] == "xx":
            V[i] = (Xc[d["a"]] * Xc[d["b"]]).astype(bf)
        elif d["op"] == "gx":
            cp = -d["ap"]
            V[i] = ((V[d["p"]].astype(np.float32) + cp) * Xc[d["c"]]).astype(bf)
        elif d["op"] == "gg":
            vp = V[d["p"]].astype(np.float32)
            vq = V[d["q"]].astype(np.float32)
            if d["aq"] == -1 or d["ap"] == -1:
                cp = -d["ap"]
                t = ((vp + cp) * vq).astype(bf).astype(np.float32)
                V[i] = ((vp + cp) + t).astype(bf)
            else:
                t = ((vp - 1.0) * vq).astype(bf).astype(np.float32)
                V[i] = ((t + 1.0) - vp).astype(bf)
    wneg = plan["wneg_bf16"].astype(np.float32)
    out = V.astype(np.float32).T @ wneg + plan["bias_f32"][None, :]
    return out


def _build_bass_kernel(plan, n_loc=N_LOC, nsub=NSUB, sim_safe=False):
    import concourse.bacc as bacc
    import concourse.tile as tile
    import concourse.mybir as mybir
    from concourse import masks

    f32 = mybir.dt.float32
    bf16 = mybir.dt.bfloat16
    mult = mybir.AluOpType.mult
    add = mybir.AluOpType.add
    subtract = mybir.AluOpType.subtract

    n_st = n_loc // (128 * nsub)
    assert n_st * 128 * nsub == n_loc

    nc = bacc.Bacc(None, target_bir_lowering=False)
    # x pre-transposed on host to slot-major bf16:
    # xg[st, p, c, j] = X[st*128*nsub + p*nsub + j, c]
    x_d = nc.dram_tensor("xg", [n_st, 128, N_FEATURES, nsub], bf16,
                         kind="ExternalInput")
    wneg2_d = nc.dram_tensor("wneg2", [2 * N_GATES, 2 * N_OUTPUTS], bf16,
                             kind="ExternalInput")
    bias_d = nc.dram_tensor("bias48", [48, 1], f32, kind="ExternalInput")
    # transposed output: row (jj*8 + o), col = (st, pair, p); host decodes
    out_d = nc.dram_tensor("out", [2 * N_OUTPUTS, n_loc // 2], f32,
                           kind="ExternalOutput")

    outr = out_d.rearrange("r (s q pp) -> s r q pp",
                           s=n_st, q=nsub // 2, pp=128)

    with tile.TileContext(nc) as tc:
        with (
            tc.tile_pool(name="const", bufs=1) as cpool,
            tc.tile_pool(name="xp", bufs=2) as xpool,
            tc.tile_pool(name="vp", bufs=2) as vpool,
            tc.tile_pool(name="tp", bufs=4) as tpool,
            tc.tile_pool(name="vs", bufs=6) as vspool,
            tc.tile_pool(name="stg", bufs=3) as stgpool,
            tc.tile_pool(name="pt", bufs=4, space="PSUM") as ptpool,
            tc.tile_pool(name="po", bufs=2, space="PSUM") as popool,
        ):
            ident = cpool.tile([128, 128], bf16)
            masks.make_identity(nc, ident[:])
            wneg2_sb = cpool.tile([2 * N_GATES, 2 * N_OUTPUTS], bf16)
            nc.sync.dma_start(wneg2_sb[:], wneg2_d[:])
            bias_sb = cpool.tile([48, 1], f32)
            nc.sync.dma_start(bias_sb[:], bias_d[:])

            for st in range(n_st):
                # x: [p, col, pair, jj] slot-major bf16 (host-transposed);
                # v: [p, pair, (gate*2 + jj)] so each pair-block is
                # contiguous (1-free-dim transpose input)
                xt = xpool.tile([128, N_FEATURES, nsub // 2, 2], bf16)
                for k in range(2):
                    nc.sync.dma_start(
                        xt[:, k * 32:(k + 1) * 32, :, :],
                        x_d[st, :, k * 32:(k + 1) * 32, :].rearrange(
                            "p c (q t) -> p c q t", t=2))
                vt = vpool.tile([128, nsub // 2, 2 * N_GATES], bf16)

                def xcol(c):
                    return xt[:, c, :, :]

                def vslot(i):
                    return vt[:, :, 2 * i:2 * i + 2]

                for d in plan["ops"]:
                    i = d["i"]
                    o = d["op"]
                    if o == "zero":
                        nc.vector.memset(vslot(i), 0.0)
                    elif o == "copyx":
                        nc.vector.tensor_copy(vslot(i), xcol(d["c"]))
                    elif o == "g1":
                        nc.scalar.activation(
                            vslot(i), vslot(d["p"]),
                            mybir.ActivationFunctionType.Identity,
                            bias=1.0, scale=float(-d["ap"]))
                    elif o == "xx":
                        # stt form measures faster than plain TENSOR_TENSOR
                        nc.vector.scalar_tensor_tensor(
                            vslot(i), xcol(d["a"]), 1.0,
                            xcol(d["b"]), mult, mult)
                    elif o == "gx":
                        nc.vector.scalar_tensor_tensor(
                            vslot(i), vslot(d["p"]), float(-d["ap"]),
                            xcol(d["c"]), add, mult)
                    elif o == "gg":
                        t = tpool.tile([128, nsub // 2, 2], bf16, tag="ggtmp")
                        if d["aq"] == -1 or d["ap"] == -1:
                            cp = float(-d["ap"])
                            nc.vector.scalar_tensor_tensor(
                                t[:], vslot(d["p"]), cp, vslot(d["q"]),
                                add, mult)
                            nc.vector.scalar_tensor_tensor(
                                vslot(i), vslot(d["p"]), cp, t[:],
                                add, add)
                        else:
                            nc.vector.scalar_tensor_tensor(
                                t[:], vslot(d["p"]), -1.0, vslot(d["q"]),
                                add, mult)
                            nc.vector.scalar_tensor_tensor(
                                vslot(i), t[:], 1.0, vslot(d["p"]),
                                add, subtract)

                # output: transpose subtile PAIRS ([128, 64g x 2j] input ->
                # [128 rows=(g,jj), 128 samples]) and project with the
                # block-diagonal wneg2 (constant stationary).  PE lhsT/rhs
                # stay at base partition 0 (base-64 operands flap the PE
                # tile config and crash at scale); projection OUTPUTS pack
                # two groups per psum tile at partition offsets {0, 32} so
                # one fused bias+drain covers 16 pairs.  Host
                # de-interleaves the [16, n] transposed output.
                for big in range(nsub // 32):     # 16 pairs per iteration
                    stg = stgpool.tile([48, 1024], f32)
                    po = popool.tile([48, 1024], f32)
                    for g8 in range(2):           # 8 pairs per pt bank
                        pt = ptpool.tile([128, 1024], bf16)
                        for c in range(8):
                            pr = big * 16 + g8 * 8 + c
                            nc.tensor.transpose(
                                pt[:, c * 128:(c + 1) * 128],
                                vt[:, pr, :], ident[:])
                        vs = vspool.tile([128, 1024], bf16)
                        nc.scalar.copy(vs[:], pt[:])
                        for c in range(2):
                            nc.tensor.matmul(
                                po[32 * g8:32 * g8 + 16,
                                   c * 512:(c + 1) * 512],
                                wneg2_sb[:], vs[:, c * 512:(c + 1) * 512],
                                start=True, stop=True)
                    # drain + bias (per-partition scalar) fused, 16 pairs.
                    # The single [48, ...] op reads the unwritten psum gap
                    # rows 16-31 (never DMA'd, benign on HW); CoreSim
                    # flags uninitialized reads, so sim builds drain the
                    # two written slices instead.
                    drains = ([(stg[:], po[:], bias_sb[:, 0:1])]
                              if not sim_safe else
                              [(stg[0:16, :], po[0:16, :], bias_sb[0:16, 0:1]),
                               (stg[32:48, :], po[32:48, :],
                                bias_sb[32:48, 0:1])])
                    for sslice, pslice, bslice in drains:
                        if big % 4 == 0:
                            nc.vector.tensor_scalar(
                                sslice, pslice, bslice, None, add)
                        else:
                            nc.scalar.activation(
                                sslice, pslice,
                                mybir.ActivationFunctionType.Identity,
                                bias=bslice, scale=1.0)
                    for g8 in range(2):
                        nc.sync.dma_start(
                            outr[st, :,
                                 big * 16 + g8 * 8:big * 16 + g8 * 8 + 8, :],
                            stg[32 * g8:32 * g8 + 16, :])

    nc.compile()
    return nc


_CACHE = {}


def _get_compiled(gate_weights, output_weights, output_scale):
    key = hashlib.sha256(
        np.asarray(gate_weights, np.float32).tobytes()
        + np.asarray(output_weights, np.float32).tobytes()
        + np.asarray(output_scale, np.float32).tobytes()
    ).hexdigest()
    if key not in _CACHE:
        plan = _build_plan(gate_weights, output_weights, output_scale)
        nc = _build_bass_kernel(plan)
        _CACHE[key] = (plan, nc)
    return _CACHE[key]


def _decode_out(dev_out, plan, n_loc=N_LOC, nsub=NSUB):
    """[16, n_loc//2] transposed device output (bias included) ->
    [n_loc, 8]."""
    n_st = n_loc // (128 * nsub)
    o5 = np.asarray(dev_out).reshape(2, N_OUTPUTS, n_st, nsub // 2, 128)
    # [jj, o, st, pr, p] -> [st, p, pr, jj, o]
    return np.transpose(o5, (2, 4, 3, 0, 1)).reshape(n_loc, N_OUTPUTS)


def make_in_maps(X, plan, n_loc=N_LOC, nsub=NSUB, n_cores=N_CORES):
    bias16 = np.concatenate([plan["bias_f32"], plan["bias_f32"]])
    bias48 = np.zeros((48, 1), dtype=np.float32)
    bias48[0:16, 0] = bias16
    bias48[32:48, 0] = bias16
    n_st = n_loc // (128 * nsub)
    # slot-major bf16: xg[core][st, p, c, j] = X[...]
    xg = (X[:n_cores * n_loc]
          .reshape(n_cores, n_st, 128, nsub, N_FEATURES)
          .transpose(0, 1, 2, 4, 3)
          .astype(ml_dtypes.bfloat16))
    in_maps = []
    for c in range(n_cores):
        in_maps.append({
            "xg": np.ascontiguousarray(xg[c]),
            "wneg2": plan["wneg2_bf16"],
            "bias48": bias48,
        })
    return in_maps


def kernel(X, gate_weights, output_weights, output_scale):
    X = np.asarray(X, dtype=np.float32)
    plan, nc = _get_compiled(gate_weights, output_weights, output_scale)
    in_maps = make_in_maps(X, plan)

    from concourse.bass_utils import run_bass_kernel_spmd
    res = run_bass_kernel_spmd(nc, in_maps, list(range(N_CORES)))
    out = np.concatenate(
        [_decode_out(res.results[c]["out"], plan) for c in range(N_CORES)],
        axis=0)
    return out.astype(np.float32)


# revision 50
# speedup vs baseline: 2.1810x; 1.0602x over previous
"""Trainium2 Bass kernel for nn_CircuitBuilder (topk_masking).

Computes, for X [524288, 64] (f32), gate_weights [64, 130], output_weights
[64, 8], output_scale [8]:

    buf = [X | 0 | 1 | gate slots]
    top2[i] = top-2 of softmax(gate_weights[i, :66+i])   (data-independent
              of X; softmax is monotonic so = top-2 of masked logits)
    g_i = 1 - a*b  (continuous NAND chain, a/b gathered from buf)
    out = (gate_matrix @ output_weights) * output_scale

Strategy (pure data parallel over 8 NeuronCores, 65536 samples each):
  - The gate wiring is computed on host from gate_weights (tiny); the
    device kernel is built for that wiring.
  - Per-core layout: partition p owns 512 consecutive samples, processed
    as 2 supertiles of nsub=256 samples per partition. X tile is
    [128, nsub, 64] f32 (contiguous DMA); gate values live in a
    [128, 64, nsub] bf16 tile ("v" storage), where slot i holds
    v_i = alpha_i * (a_i*b_i) with a per-gate sign alpha chosen so each
    gate needs 1 fused DVE op (2 for gate×gate gates):
        m_i = a*b,  g_i = 1 - m_i,  v_i = alpha_i * m_i
    using scalar_tensor_tensor  out = (in0 op0 scalar) op1 in1.
  - Output: out = bias - sum_i W_i*m_i = bias + sum_i Wneg_i*v_i with
    Wneg_i = -alpha_i*W_i*scale. The v tile is rotated per 128-sample
    subtile with TensorE transposes into PSUM (packed bf16), drained to
    SBUF, then matmul'd against Wneg into [128, 8] psum chunks, bias
    added and DMA'd out.
"""

import hashlib
import sys
import types

import numpy as np
import ml_dtypes

N_SAMPLES = 524288
N_FEATURES = 64
N_GATES = 64
N_OUTPUTS = 8
BASE = N_FEATURES + 2            # 66
MAX_CONN = BASE + N_GATES        # 130
N_CORES = 8
N_LOC = N_SAMPLES // N_CORES     # 65536 samples per core
NSUB = 256                       # samples per partition per supertile
N_ST = N_LOC // (128 * NSUB)     # supertiles per core (2)


def _top2(gate_weights: np.ndarray) -> np.ndarray:
    """Top-2 connection indices per gate (matches jax.lax.top_k of the
    softmax: softmax is monotonic, top_k ties break to lower index,
    stable argsort of the negated row reproduces that)."""
    top2 = np.zeros((N_GATES, 2), dtype=np.int64)
    for i in range(N_GATES):
        row = np.asarray(gate_weights[i], dtype=np.float32).copy()
        row[BASE + i:] = -1e9
        top2[i] = np.argsort(-row, kind="stable")[:2]
    return top2


def _build_plan(gate_weights, output_weights, output_scale):
    """Host-side gate wiring -> per-gate op descriptors + output weights."""
    top2 = _top2(gate_weights)
    W = np.asarray(output_weights, dtype=np.float64)
    scale = np.asarray(output_scale, dtype=np.float64)

    ops = []          # list of dicts describing device ops per gate
    alpha = np.zeros(N_GATES, dtype=np.int64)
    for i in range(N_GATES):
        c0, c1 = int(top2[i][0]), int(top2[i][1])

        def kind(c):
            if c < N_FEATURES:
                return "x"
            if c == N_FEATURES:
                return "0"
            if c == N_FEATURES + 1:
                return "1"
            return "g"

        k0, k1 = kind(c0), kind(c1)
        # order canonically: g-operands first, then x, consts last
        pri = {"g": 0, "x": 1, "1": 2, "0": 3}
        if pri[k0] > pri[k1]:
            c0, c1, k0, k1 = c1, c0, k1, k0
        kk = k0 + k1
        if "0" in kk:
            ops.append({"op": "zero", "i": i})
            alpha[i] = 1
        elif kk == "x1":
            ops.append({"op": "copyx", "i": i, "c": c0})
            alpha[i] = 1
        elif kk == "g1":
            p = c0 - BASE
            ops.append({"op": "g1", "i": i, "p": p, "ap": int(alpha[p])})
            alpha[i] = 1
        elif kk == "xx":
            ops.append({"op": "xx", "i": i, "a": c0, "b": c1})
            alpha[i] = 1
        elif kk == "gx":
            p = c0 - BASE
            ap = int(alpha[p])
            ops.append({"op": "gx", "i": i, "p": p, "c": c1, "ap": ap})
            alpha[i] = -ap
        elif kk == "gg":
            p, q = c0 - BASE, c1 - BASE
            ap, aq = int(alpha[p]), int(alpha[q])
            if aq != -1 and ap == -1:
                p, q, ap, aq = q, p, aq, ap
            ops.append({"op": "gg", "i": i, "p": p, "q": q, "ap": ap, "aq": aq})
            alpha[i] = -ap if (aq == -1) else 1
        else:
            raise AssertionError(f"unexpected operand kinds {kk}")

    wneg = (-alpha[:, None] * W * scale[None, :]).astype(np.float64)
    bias = (W * scale[None, :]).sum(axis=0)
    # block-diagonal-by-subtile-parity projection matrix for the
    # pair-transpose scheme: row (g*2 + jj), col (jj*8 + o) = wneg[g, o]
    wneg2 = np.zeros((2 * N_GATES, 2 * N_OUTPUTS), dtype=np.float64)
    for jj in range(2):
        wneg2[jj::2, jj * N_OUTPUTS:(jj + 1) * N_OUTPUTS] = wneg
    return {
        "ops": ops,
        "alpha": alpha,
        "wneg_bf16": wneg.astype(ml_dtypes.bfloat16),
        "wneg2_bf16": wneg2.astype(ml_dtypes.bfloat16),
        "bias_f32": bias.astype(np.float32),
        "top2": top2,
    }


def _emulate_plan(plan, X):
    """Numpy emulation of the device program (bf16 v-storage) — used for
    host-side self-checks in development."""
    n = X.shape[0]
    bf = ml_dtypes.bfloat16
    V = np.zeros((N_GATES, n), dtype=bf)
    # device receives X pre-quantized to bf16
    Xc = np.asarray(X, dtype=np.float32).T.astype(bf).astype(np.float32)
    for d in plan["ops"]:
        i = d["i"]
        if d["op"] == "zero":
            V[i] = 0
        elif d["op"] == "copyx":
            V[i] = Xc[d["c"]].astype(bf)
        elif d["op"] == "g1":
            V[i] = (V[d["p"]].astype(np.float32) * (-d["ap"]) + 1.0).astype(bf)
        elif d["op"] == "xx":
            V[i] = (Xc[d["a"]] * Xc[d["b"]]).astype(bf)
        elif d["op"] == "gx":
            cp = -d["ap"]
            V[i] = ((V[d["p"]].astype(np.float32) + cp) * Xc[d["c"]]).astype(bf)
        elif d["op"] == "gg":
            vp = V[d["p"]].astype(np.float32)
            vq = V[d["q"]].astype(np.float32)
            if d["aq"] == -1 or d["ap"] == -1:
                cp = -d["ap"]
                t = ((vp + cp) * vq).astype(bf).astype(np.float32)
                V[i] = ((vp + cp) + t).astype(bf)
            else:
                t = ((vp - 1.0) * vq).astype(bf).astype(np.float32)
                V[i] = ((t + 1.0) - vp).astype(bf)
    wneg = plan["wneg_bf16"].astype(np.float32)
    out = V.astype(np.float32).T @ wneg + plan["bias_f32"][None, :]
    return out


def _build_bass_kernel(plan, n_loc=N_LOC, nsub=NSUB, sim_safe=False):
    import concourse.bacc as bacc
    import concourse.tile as tile
    import concourse.mybir as mybir
    from concourse import masks

    f32 = mybir.dt.float32
    bf16 = mybir.dt.bfloat16
    mult = mybir.AluOpType.mult
    add = mybir.AluOpType.add
    subtract = mybir.AluOpType.subtract

    n_st = n_loc // (128 * nsub)
    assert n_st * 128 * nsub == n_loc

    nc = bacc.Bacc(None, target_bir_lowering=False)
    # x pre-transposed on host to slot-major bf16:
    # xg[st, p, c, j] = X[st*128*nsub + p*nsub + j, c]
    x_d = nc.dram_tensor("xg", [n_st, 128, N_FEATURES, nsub], bf16,
                         kind="ExternalInput")
    wneg2_d = nc.dram_tensor("wneg2", [2 * N_GATES, 2 * N_OUTPUTS], bf16,
                             kind="ExternalInput")
    bias_d = nc.dram_tensor("bias48", [48, 1], f32, kind="ExternalInput")
    # transposed output: row (jj*8 + o), col = (st, pair, p); host decodes
    out_d = nc.dram_tensor("out", [2 * N_OUTPUTS, n_loc // 2], f32,
                           kind="ExternalOutput")

    outr = out_d.rearrange("r (s q pp) -> s r q pp",
                           s=n_st, q=nsub // 2, pp=128)

    with tile.TileContext(nc) as tc:
        with (
            tc.tile_pool(name="const", bufs=1) as cpool,
            tc.tile_pool(name="xp", bufs=2) as xpool,
            tc.tile_pool(name="vp", bufs=2) as vpool,
            tc.tile_pool(name="tp", bufs=4) as tpool,
            tc.tile_pool(name="vs", bufs=6) as vspool,
            tc.tile_pool(name="stg", bufs=3) as stgpool,
            tc.tile_pool(name="pt", bufs=4, space="PSUM") as ptpool,
            tc.tile_pool(name="po", bufs=2, space="PSUM") as popool,
        ):
            ident = cpool.tile([128, 128], bf16)
            masks.make_identity(nc, ident[:])
            wneg2_sb = cpool.tile([2 * N_GATES, 2 * N_OUTPUTS], bf16)
            nc.sync.dma_start(wneg2_sb[:], wneg2_d[:])
            bias_sb = cpool.tile([48, 1], f32)
            nc.sync.dma_start(bias_sb[:], bias_d[:])

            for st in range(n_st):
                # x: [p, col, pair, jj] slot-major bf16 (host-transposed);
                # v: [p, pair, (gate*2 + jj)] so each pair-block is
                # contiguous (1-free-dim transpose input)
                xt = xpool.tile([128, N_FEATURES, nsub // 2, 2], bf16)
                for k in range(2):
                    nc.sync.dma_start(
                        xt[:, k * 32:(k + 1) * 32, :, :],
                        x_d[st, :, k * 32:(k + 1) * 32, :].rearrange(
                            "p c (q t) -> p c q t", t=2))
                vt = vpool.tile([128, nsub // 2, 2 * N_GATES], bf16)

                def xcol(c):
                    return xt[:, c, :, :]

                def vslot(i):
                    return vt[:, :, 2 * i:2 * i + 2]

                for d in plan["ops"]:
                    i = d["i"]
                    o = d["op"]
                    if o == "zero":
                        nc.vector.memset(vslot(i), 0.0)
                    elif o == "copyx":
                        nc.vector.tensor_copy(vslot(i), xcol(d["c"]))
                    elif o == "g1":
                        nc.scalar.activation(
                            vslot(i), vslot(d["p"]),
                            mybir.ActivationFunctionType.Identity,
                            bias=1.0, scale=float(-d["ap"]))
                    elif o == "xx":
                        # stt form measures faster than plain TENSOR_TENSOR
                        nc.vector.scalar_tensor_tensor(
                            vslot(i), xcol(d["a"]), 1.0,
                            xcol(d["b"]), mult, mult)
                    elif o == "gx":
                        nc.vector.scalar_tensor_tensor(
                            vslot(i), vslot(d["p"]), float(-d["ap"]),
                            xcol(d["c"]), add, mult)
                    elif o == "gg":
                        t = tpool.tile([128, nsub // 2, 2], bf16, tag="ggtmp")
                        if d["aq"] == -1 or d["ap"] == -1:
                            cp = float(-d["ap"])
                            nc.vector.scalar_tensor_tensor(
                                t[:], vslot(d["p"]), cp, vslot(d["q"]),
                                add, mult)
                            nc.vector.scalar_tensor_tensor(
                                vslot(i), vslot(d["p"]), cp, t[:],
                                add, add)
                        else:
                            nc.vector.scalar_tensor_tensor(
                                t[:], vslot(d["p"]), -1.0, vslot(d["q"]),
                                add, mult)
                            nc.vector.scalar_tensor_tensor(
                                vslot(i), t[:], 1.0, vslot(d["p"]),
                                add, subtract)

                # output: transpose subtile PAIRS ([128, 64g x 2j] input ->
                # [128 rows=(g,jj), 128 samples]) and project with the
                # block-diagonal wneg2 (constant stationary).  PE lhsT/rhs
                # stay at base partition 0 (base-64 operands flap the PE
                # tile config and crash at scale); projection OUTPUTS pack
                # two groups per psum tile at partition offsets {0, 32} so
                # one fused bias+drain covers 16 pairs.  Host
                # de-interleaves the [16, n] transposed output.
                for big in range(nsub // 32):     # 16 pairs per iteration
                    stg = stgpool.tile([48, 1024], f32)
                    po = popool.tile([48, 1024], f32)
                    for g8 in range(2):           # 8 pairs per pt bank
                        pt = ptpool.tile([128, 1024], bf16)
                        for c in range(8):
                            pr = big * 16 + g8 * 8 + c
                            nc.tensor.transpose(
                                pt[:, c * 128:(c + 1) * 128],
                                vt[:, pr, :], ident[:])
                        vs = vspool.tile([128, 1024], bf16)
                        # last supertile's output phase is the pipeline
                        # tail: DVE is idle there, so share its drains
                        if st == n_st - 1 and g8 % 2 == 0:
                            nc.vector.tensor_copy(vs[:], pt[:])
                        else:
                            nc.scalar.copy(vs[:], pt[:])
                        for c in range(2):
                            nc.tensor.matmul(
                                po[32 * g8:32 * g8 + 16,
                                   c * 512:(c + 1) * 512],
                                wneg2_sb[:], vs[:, c * 512:(c + 1) * 512],
                                start=True, stop=True)
                    # drain + bias (per-partition scalar) fused, 16 pairs.
                    # The single [48, ...] op reads the unwritten psum gap
                    # rows 16-31 (never DMA'd, benign on HW); CoreSim
                    # flags uninitialized reads, so sim builds drain the
                    # two written slices instead.
                    drains = ([(stg[:], po[:], bias_sb[:, 0:1])]
                              if not sim_safe else
                              [(stg[0:16, :], po[0:16, :], bias_sb[0:16, 0:1]),
                               (stg[32:48, :], po[32:48, :],
                                bias_sb[32:48, 0:1])])
                    for sslice, pslice, bslice in drains:
                        if (big % 2 == 0) if st == n_st - 1 else False:
                            nc.vector.tensor_scalar(
                                sslice, pslice, bslice, None, add)
                        else:
                            nc.scalar.activation(
                                sslice, pslice,
                                mybir.ActivationFunctionType.Identity,
                                bias=bslice, scale=1.0)
                    for g8 in range(2):
                        nc.sync.dma_start(
                            outr[st, :,
                                 big * 16 + g8 * 8:big * 16 + g8 * 8 + 8, :],
                            stg[32 * g8:32 * g8 + 16, :])

    nc.compile()
    return nc


_CACHE = {}


def _get_compiled(gate_weights, output_weights, output_scale):
    key = hashlib.sha256(
        np.asarray(gate_weights, np.float32).tobytes()
        + np.asarray(output_weights, np.float32).tobytes()
        + np.asarray(output_scale, np.float32).tobytes()
    ).hexdigest()
    if key not in _CACHE:
        plan = _build_plan(gate_weights, output_weights, output_scale)
        nc = _build_bass_kernel(plan)
        _CACHE[key] = (plan, nc)
    return _CACHE[key]


def _decode_out(dev_out, plan, n_loc=N_LOC, nsub=NSUB):
    """[16, n_loc//2] transposed device output (bias included) ->
    [n_loc, 8]."""
    n_st = n_loc // (128 * nsub)
    o5 = np.asarray(dev_out).reshape(2, N_OUTPUTS, n_st, nsub // 2, 128)
    # [jj, o, st, pr, p] -> [st, p, pr, jj, o]
    return np.transpose(o5, (2, 4, 3, 0, 1)).reshape(n_loc, N_OUTPUTS)


def make_in_maps(X, plan, n_loc=N_LOC, nsub=NSUB, n_cores=N_CORES):
    bias16 = np.concatenate([plan["bias_f32"], plan["bias_f32"]])
    bias48 = np.zeros((48, 1), dtype=np.float32)
    bias48[0:16, 0] = bias16
    bias48[32:48, 0] = bias16
    n_st = n_loc // (128 * nsub)
    # slot-major bf16: xg[core][st, p, c, j] = X[...]
    xg = (X[:n_cores * n_loc]
          .reshape(n_cores, n_st, 128, nsub, N_FEATURES)
          .transpose(0, 1, 2, 4, 3)
          .astype(ml_dtypes.bfloat16))
    in_maps = []
    for c in range(n_cores):
        in_maps.append({
            "xg": np.ascontiguousarray(xg[c]),
            "wneg2": plan["wneg2_bf16"],
            "bias48": bias48,
        })
    return in_maps


def kernel(X, gate_weights, output_weights, output_scale):
    X = np.asarray(X, dtype=np.float32)
    plan, nc = _get_compiled(gate_weights, output_weights, output_scale)
    in_maps = make_in_maps(X, plan)

    from concourse.bass_utils import run_bass_kernel_spmd
    res = run_bass_kernel_spmd(nc, in_maps, list(range(N_CORES)))
    out = np.concatenate(
        [_decode_out(res.results[c]["out"], plan) for c in range(N_CORES)],
        axis=0)
    return out.astype(np.float32)
